# revision 17
# baseline (speedup 1.0000x reference)
"""Trainium2 Bass kernel for nn_Bilinear_70222715290053.

Problem: x [128, 224, 224, 5] f32 where channels 0:3 are an image and
channels 3,4 are per-pixel displacements (dx, dy). Output [128,224,224,3]:
  out[b,i,j,:] = img[b, int(mod(i+dy, 224)), int(mod(j+dx, 224)), :]

Key property: dx, dy ~ N(0,1), so |displacement| <= ~5.5 — the gather is a
LOCAL warp within a 13x13 window (kernel() verifies the bound at runtime
and falls back to an exact path if violated).

System design (the host<->device axon tunnel runs at ~40MB/s and is the
end-to-end bottleneck, so minimize bytes moved):
  - Host side (cheap, vectorized, bit-exact vs the CPU jax reference —
    verified): compute the source-pixel window offsets and fold them into
    one uint8 code = (offy+6)*13 + (offx+6) per pixel [6.4MB], and convert
    the 3 image channels to bf16 [38.6MB]. Upload 45MB instead of 128MB.
  - Device side (Bass, SPMD on 8 cores, batch-sharded 16 images/core):
    rows live in the partition dim; a round processes G=4 images x 112
    output rows (+6 halo rows each side via extra partitions, mod-224
    wrapped). 169 select terms: mask = is_equal(code, t) on the DVE, then
    3 copy_predicated moves from the (oy, ox)-shifted x-padded channel
    planes into the output tile. Partition shifts are materialized by
    cheap SBUF->SBUF DMAs (compute engines can only address partitions
    0/32/64/96; DMA has no such limit). Every pixel matches exactly one
    term.
  - Output returns as bf16 [38.6MB instead of 77MB]; host upcasts to f32.
    Total quantization error is one bf16 rounding: rel err ~4e-3 (< 2e-2).

Self-contained: builds the Bass module, compiles through neuronx_cc via the
bass2jax custom call, and runs SPMD on 8 NeuronCores via shard_map.
"""

import sys

sys.path.insert(0, "/opt/trn_rl_repo")

import numpy as np

_CACHE = {}

_B, _H, _W = 16, 224, 224  # per-core shard
_HALO = 6
_WIN = 2 * _HALO + 1
_G = 4  # images per round


def _build_module(B=_B, H=_H, W=_W, G=_G, HALO=_HALO):
    from concourse import mybir, bacc
    import concourse.tile as tile

    BF16 = mybir.dt.bfloat16
    F16 = mybir.dt.float16
    I16 = mybir.dt.int16
    U8 = mybir.dt.uint8
    Alu = mybir.AluOpType

    NB = 2               # row blocks per image
    RB = H // NB         # 112 output rows per block
    WPAD = W + 2 * HALO  # 236 x-padded plane width
    WIN = 2 * HALO + 1
    R3 = W * 3
    # Band partition layout: 0..RB-1 = central rows r0..r0+RB-1,
    # RB..RB+HALO-1 = top halo rows r0-HALO..r0-1 (mod H),
    # RB+HALO..RB+2*HALO-1 = bottom halo rows r0+RB..+RB+HALO-1 (mod H).
    PT = RB
    PB = RB + HALO
    BAND = RB + 2 * HALO

    nc = bacc.Bacc(None, target_bir_lowering=False)
    img = nc.declare_dram_parameter("img", [B * H, R3], BF16, isOutput=False)
    code = nc.declare_dram_parameter("code", [B * H, W], U8, isOutput=False)
    y = nc.declare_dram_parameter("y", [B * H, R3], BF16, isOutput=True)

    with tile.TileContext(nc) as tc:
        with (
            tc.tile_pool(name="rec", bufs=2) as rpool,
            tc.tile_pool(name="planes", bufs=2) as ppool,
            tc.tile_pool(name="shift", bufs=2) as spool,
            tc.tile_pool(name="tmp", bufs=2) as tpool,
            tc.tile_pool(name="outp", bufs=2) as opool,
        ):
            for g0 in range(0, B, G):
                for blk in range(NB):
                    r0 = blk * RB
                    rtop = (r0 - HALO) % H
                    rbot = (r0 + RB) % H
                    band = rpool.tile([128, G * R3], BF16, tag="band")
                    codet = tpool.tile([128, G * W], U8, tag="codet")
                    for gi in range(G):
                        b = g0 + gi
                        fs = slice(gi * R3, (gi + 1) * R3)
                        nc.sync.dma_start(
                            out=band[0:RB, fs],
                            in_=img[b * H + r0 : b * H + r0 + RB, :],
                        )
                        nc.sync.dma_start(
                            out=band[PT : PT + HALO, fs],
                            in_=img[b * H + rtop : b * H + rtop + HALO, :],
                        )
                        nc.sync.dma_start(
                            out=band[PB : PB + HALO, fs],
                            in_=img[b * H + rbot : b * H + rbot + HALO, :],
                        )
                        nc.sync.dma_start(
                            out=codet[0:RB, gi * W : (gi + 1) * W],
                            in_=code[b * H + r0 : b * H + r0 + RB, :],
                        )

                    band4 = band[:].rearrange("p (g w c) -> p g w c", g=G, c=3)

                    # x-wrap-padded bf16 channel planes [BAND, G, WPAD]
                    planes = []
                    for c in range(3):
                        pc = ppool.tile([128, G * WPAD], BF16, tag=f"plane{c}")
                        pc3 = pc[:].rearrange("p (g w) -> p g w", g=G)
                        nc.vector.tensor_copy(
                            out=pc3[0:BAND, :, HALO : HALO + W],
                            in_=band4[0:BAND, :, :, c : c + 1].rearrange(
                                "p g w k -> p g (w k)"
                            ),
                        )
                        nc.vector.tensor_copy(
                            out=pc3[0:BAND, :, 0:HALO],
                            in_=band4[
                                0:BAND, :, W - HALO : W, c : c + 1
                            ].rearrange("p g w k -> p g (w k)"),
                        )
                        nc.vector.tensor_copy(
                            out=pc3[0:BAND, :, HALO + W : WPAD],
                            in_=band4[0:BAND, :, 0:HALO, c : c + 1].rearrange(
                                "p g w k -> p g (w k)"
                            ),
                        )
                        planes.append(pc)

                    def s(t):
                        return t[0:RB, :]

                    # code as fp16 so the 169 is_equal passes run in the
                    # DVE 16-bit fast mode
                    codeh = tpool.tile([128, G * W], F16, tag="codeh")
                    nc.vector.tensor_copy(out=s(codeh), in_=s(codet))

                    # Pad per-image strides of mask/out so their interp views
                    # cannot dim-merge (copy_predicated needs all three
                    # operand views shaped identically (RB, G, W)).
                    OSTR = R3 + 16
                    MSTR = W + 16
                    out_t = opool.tile([128, G * OSTR], BF16, tag="out")
                    out4 = (
                        out_t[:]
                        .rearrange("p (g q) -> p g q", g=G)[:, :, 0:R3]
                        .rearrange("p g (w c) -> p g w c", c=3)
                    )
                    maskh = tpool.tile([128, G * MSTR], I16, tag="maskh")
                    mview = maskh[:].rearrange("p (g q) -> p g q", g=G)[
                        0:RB, :, 0:W
                    ]
                    ch3 = codeh[:].rearrange("p (g w) -> p g w", g=G)

                    for oy in range(-HALO, HALO + 1):
                        # partition-shifted plane copies (DMA may start at
                        # any partition; compute engines may not). Band
                        # layout makes each shift at most 2 contiguous
                        # pieces.
                        if oy == 0:
                            sps = planes
                        else:
                            sps = []
                            for c in range(3):
                                sp = spool.tile(
                                    [128, G * WPAD], BF16, tag=f"sp{c}"
                                )
                                if oy < 0:
                                    k = -oy
                                    nc.sync.dma_start(
                                        out=sp[0:k, :],
                                        in_=planes[c][PB - k : PB, :],
                                    )
                                    nc.sync.dma_start(
                                        out=sp[k:RB, :],
                                        in_=planes[c][0 : RB - k, :],
                                    )
                                else:
                                    nc.sync.dma_start(
                                        out=sp[0 : RB - oy, :],
                                        in_=planes[c][oy:RB, :],
                                    )
                                    nc.sync.dma_start(
                                        out=sp[RB - oy : RB, :],
                                        in_=planes[c][PB : PB + oy, :],
                                    )
                                sps.append(sp)
                        spv = [
                            t[:].rearrange("p (g w) -> p g w", g=G) for t in sps
                        ]
                        for ox in range(-HALO, HALO + 1):
                            t_code = float((oy + HALO) * WIN + (ox + HALO))
                            nc.vector.tensor_scalar(
                                out=mview, in0=ch3[0:RB],
                                scalar1=t_code, scalar2=None,
                                op0=Alu.is_equal,
                            )
                            for c in range(3):
                                nc.vector.copy_predicated(
                                    out4[0:RB, :, :, c : c + 1].rearrange(
                                        "p g w k -> p g (w k)"
                                    ),
                                    mview,
                                    spv[c][0:RB, :, HALO + ox : HALO + ox + W],
                                )

                    for gi in range(G):
                        b = g0 + gi
                        nc.sync.dma_start(
                            out=y[b * H + r0 : b * H + r0 + RB, :],
                            in_=out_t[0:RB, gi * OSTR : gi * OSTR + R3],
                        )
    return nc


def _split_multiwait_drains(nc):
    """This walrus build accepts one sync wait per Drain (TPB_CTRL); split
    the Tile epilogue's multi-wait drains into single-wait chains."""
    import copy
    import bass_rust
    from concourse import mybir

    changed = False
    new_functions = []
    for function in nc.m.functions:
        new_function = copy.replace(function, blocks=[])
        new_function.set_allocations_from_list(function.allocations)
        for block in function.blocks:
            new_insts = []
            for ins in block.instructions:
                si = ins.sync_info
                if (
                    isinstance(ins, (mybir.InstDrain, mybir.InstNoOp))
                    and si is not None
                    and len(si.on_wait) > 1
                ):
                    changed = True
                    waits = list(si.on_wait)
                    for i, w in enumerate(waits[:-1]):
                        d = mybir.InstDrain(
                            name=f"{ins.name}_sw{i}", ins=[], outs=[],
                            bass_is_fusable=False,
                        )
                        d.engine = ins.engine
                        d.sync_info = bass_rust.SyncInfo(on_wait=[w], on_update=[])
                        new_insts.append(d)
                    ins.sync_info = bass_rust.SyncInfo(
                        on_wait=[waits[-1]], on_update=list(si.on_update)
                    )
                new_insts.append(ins)
            new_function.blocks.append(copy.replace(block, instructions=new_insts))
        new_functions.append(new_function)
    if changed:
        nc.m = copy.replace(nc.m, functions=new_functions)
    return nc


class _Runner:
    def __init__(self, nc, n_cores=8):
        import jax
        from jax.sharding import Mesh, PartitionSpec, NamedSharding
        from jax.experimental.shard_map import shard_map
        from concourse import mybir
        from concourse.bass2jax import (
            _bass_exec_p,
            install_neuronx_cc_hook,
            partition_id_tensor,
        )

        install_neuronx_cc_hook()
        if not nc.is_finalized():
            nc.finalize()
        _split_multiwait_drains(nc)

        self.jax = jax
        partition_name = (
            nc.partition_id_tensor.name if nc.partition_id_tensor else None
        )
        in_names, out_names, out_avals, zero_shapes = [], [], [], []
        for alloc in nc.m.functions[0].allocations:
            if not isinstance(alloc, mybir.MemoryLocationSet):
                continue
            name = alloc.memorylocations[0].name
            if alloc.kind == "ExternalInput":
                if name != partition_name:
                    in_names.append(name)
            elif alloc.kind == "ExternalOutput":
                out_names.append(name)
                shape = tuple(alloc.tensor_shape)
                dtype = mybir.dt.np(alloc.dtype)
                out_avals.append(jax.core.ShapedArray(shape, dtype))
                zero_shapes.append((shape, dtype))
        n_params = len(in_names)
        n_outs = len(out_avals)
        all_in_names = list(in_names) + list(out_names)
        if partition_name is not None:
            all_in_names.append(partition_name)
        donate = tuple(range(n_params, n_params + n_outs))

        def _body(*args):
            operands = list(args)
            if partition_name is not None:
                operands.append(partition_id_tensor())
            outs = _bass_exec_p.bind(
                *operands,
                out_avals=tuple(out_avals),
                in_names=tuple(all_in_names),
                out_names=tuple(out_names),
                lowering_input_output_aliases=(),
                sim_require_finite=True,
                sim_require_nnan=True,
                nc=nc,
            )
            return tuple(outs)

        devices = jax.devices()[:n_cores]
        mesh = Mesh(np.asarray(devices), ("core",))
        in_specs = (PartitionSpec("core"),) * (n_params + n_outs)
        out_specs = (PartitionSpec("core"),) * n_outs
        self.sharded = jax.jit(
            shard_map(
                _body, mesh=mesh, in_specs=in_specs, out_specs=out_specs,
                check_rep=False,
            ),
            donate_argnums=donate,
            keep_unused=True,
        )
        self.devices = devices
        self.shard = NamedSharding(mesh, PartitionSpec("core"))
        self.in_names, self.out_names = in_names, out_names
        self.out_avals, self.zero_shapes = out_avals, zero_shapes
        self.n_cores = n_cores
        # y-init buffers: uploaded once, then recycled from the previous
        # call's outputs (the kernel fully overwrites y; donation consumes
        # the buffers each call)
        self._ybufs = None

    def prep_inputs(self, in_maps):
        """Upload per-core shards in parallel (one device_put per device)."""
        from concurrent.futures import ThreadPoolExecutor

        jax = self.jax
        arrays = []
        for name in self.in_names:
            shards = [np.asarray(m[name]) for m in in_maps]
            full_shape = (
                self.n_cores * shards[0].shape[0],
                *shards[0].shape[1:],
            )
            with ThreadPoolExecutor(self.n_cores) as ex:
                parts = list(
                    ex.map(
                        lambda t: jax.device_put(t[0], t[1]),
                        zip(shards, self.devices),
                    )
                )
            arrays.append(
                jax.make_array_from_single_device_arrays(
                    full_shape, self.shard, parts
                )
            )
        jax.block_until_ready(arrays)
        return arrays

    def _get_ybufs(self):
        if self._ybufs is None:
            jax = self.jax
            zs = [
                jax.device_put(
                    np.zeros((self.n_cores * s[0], *s[1:]), d), self.shard
                )
                for (s, d) in self.zero_shapes
            ]
            jax.block_until_ready(zs)
            self._ybufs = zs
        return self._ybufs

    def run(self, dev_in):
        out = self.sharded(*dev_in, *self._get_ybufs())
        self.jax.block_until_ready(out)
        self._ybufs = list(out)
        return out

    def fetch(self, out):
        """Download the output shards in parallel."""
        from concurrent.futures import ThreadPoolExecutor

        res = []
        for arr in out:
            shards = sorted(
                arr.addressable_shards, key=lambda s: s.index[0].start
            )
            with ThreadPoolExecutor(self.n_cores) as ex:
                parts = list(ex.map(lambda s: np.asarray(s.data), shards))
            res.append(parts)
        return res

    def run_maps(self, in_maps):
        parts = self.fetch(self.run(self.prep_inputs(in_maps)))
        return [
            {name: parts[i][c] for i, name in enumerate(self.out_names)}
            for c in range(self.n_cores)
        ]


def _get_runner():
    if "r" not in _CACHE:
        _CACHE["r"] = _Runner(_build_module())
    return _CACHE["r"]


def _host_code(dx, dy):
    """Window-offset code per pixel, bit-exact vs the CPU jax reference.
    Returns (code uint8, ok bool)."""
    H, W = _H, _W
    cols = np.arange(W, dtype=np.float32)[None, None, :]
    rows = np.arange(H, dtype=np.float32)[None, :, None]

    def idx(t, lim):
        t = t.astype(np.float32)
        tw = t.copy()
        neg = t < 0
        hi = t >= lim
        tw[neg] = t[neg] + np.float32(lim)
        tw[hi] = t[hi] - np.float32(lim)
        return np.minimum(tw.astype(np.int32), lim - 1)

    Xi = idx(cols + dx, W)
    Yi = idx(rows + dy, H)
    offx = Xi - np.arange(W, dtype=np.int32)[None, None, :]
    offy = Yi - np.arange(H, dtype=np.int32)[None, :, None]
    offx = np.where(offx > 112, offx - W, offx)
    offx = np.where(offx < -112, offx + W, offx)
    offy = np.where(offy > 112, offy - H, offy)
    offy = np.where(offy < -112, offy + H, offy)
    ok = bool(
        offx.min() >= -_HALO and offx.max() <= _HALO
        and offy.min() >= -_HALO and offy.max() <= _HALO
    )
    if not ok:
        return None, False
    codes = ((offy + _HALO) * _WIN + (offx + _HALO)).astype(np.uint8)
    return codes, True


def _kernel_np(x):
    """Exact reference semantics (including jax's clamp of the f32 mod
    boundary case) — robustness fallback."""
    H, W = _H, _W
    img = x[..., 0:3]
    dx = x[..., 3]
    dy = x[..., 4]
    cols = np.arange(W, dtype=np.float32)
    rows = np.arange(H, dtype=np.float32)[:, None]
    Xi = np.minimum(
        np.mod(cols[None, None, :] + dx, np.float32(W)).astype(np.int32), W - 1
    )
    Yi = np.minimum(
        np.mod(rows[None, :, :] + dy, np.float32(H)).astype(np.int32), H - 1
    )
    b = np.arange(x.shape[0])[:, None, None]
    return img[b, Yi, Xi]


def _kernel_jax_device(x):
    """Tier-2 fallback: run the warp gather on the 8 NeuronCores via
    XLA-Neuron's native gather path."""
    import jax
    import jax.numpy as jnp

    H, W = _H, _W

    def body(xs):  # [B, H, W, 5] per device
        img = xs[..., 0:3]
        dx = xs[..., 3]
        dy = xs[..., 4]
        cols = jnp.arange(W, dtype=jnp.float32)
        rows = jnp.arange(H, dtype=jnp.float32)[:, None]
        Xi = jnp.mod(cols[None, None, :] + dx, float(W)).astype(jnp.int32)
        Yi = jnp.mod(rows[None, :, :] + dy, float(H)).astype(jnp.int32)
        b = jnp.arange(xs.shape[0])[:, None, None]
        return img[b, Yi, Xi]

    if "jdk" not in _CACHE:
        _CACHE["jdk"] = jax.jit(body)
    f = _CACHE["jdk"]
    devices = jax.devices()[:8]
    shards = x.reshape(8, _B, H, W, 5)
    dev_in = [jax.device_put(shards[i], devices[i]) for i in range(8)]
    outs = [f(s) for s in dev_in]
    host = jax.device_get(outs)
    return np.concatenate(host, axis=0)


_USE_BASS = True


def kernel(x):
    import ml_dtypes

    x = np.ascontiguousarray(np.asarray(x, dtype=np.float32))
    assert x.shape == (128, _H, _W, 5), x.shape
    n_cores = 8
    if _USE_BASS:
        try:
            codes, ok = _host_code(x[..., 3], x[..., 4])
            if ok:
                img16 = np.ascontiguousarray(x[..., 0:3]).astype(
                    ml_dtypes.bfloat16
                )
                img_sh = img16.reshape(n_cores, _B * _H, _W * 3)
                code_sh = codes.reshape(n_cores, _B * _H, _W)
                in_maps = [
                    {"img": img_sh[c], "code": code_sh[c]}
                    for c in range(n_cores)
                ]
                outs = _get_runner().run_maps(in_maps)
                y = np.stack([o["y"] for o in outs])  # [8, B*H, W*3] bf16
                return y.astype(np.float32).reshape(128, _H, _W, 3)
        except Exception as e:
            sys.stderr.write(
                f"kernel: bass path failed ({e!r}); jax-device fallback\n"
            )
    try:
        return _kernel_jax_device(x)
    except Exception as e:
        sys.stderr.write(f"kernel: jax-device failed ({e!r}); numpy fallback\n")
        return _kernel_np(x)


# revision 19
# speedup vs baseline: 1.2374x; 1.2374x over previous
"""Trainium2 Bass kernel for nn_Bilinear_70222715290053.

Problem: x [128, 224, 224, 5] f32 where channels 0:3 are an image and
channels 3,4 are per-pixel displacements (dx, dy). Output [128,224,224,3]:
  out[b,i,j,:] = img[b, int(mod(i+dy, 224)), int(mod(j+dx, 224)), :]

Key property: dx, dy ~ N(0,1), so |displacement| <= ~5.5 — the gather is a
LOCAL warp within a 13x13 window (kernel() verifies the bound at runtime
and falls back to an exact path if violated).

System design (the host<->device axon tunnel runs at ~40MB/s and is the
end-to-end bottleneck, so minimize bytes moved):
  - Host side (cheap, vectorized, bit-exact vs the CPU jax reference —
    verified): compute the source-pixel window offsets and fold them into
    one uint8 code = (offy+6)*13 + (offx+6) per pixel [6.4MB], and convert
    the 3 image channels to bf16 [38.6MB]. Upload 45MB instead of 128MB.
  - Device side (Bass, SPMD on 8 cores, batch-sharded 16 images/core):
    rows live in the partition dim; a round processes G=4 images x 112
    output rows (+6 halo rows each side via extra partitions, mod-224
    wrapped). 169 select terms: mask = is_equal(code, t) on the DVE, then
    3 copy_predicated moves from the (oy, ox)-shifted x-padded channel
    planes into the output tile. Partition shifts are materialized by
    cheap SBUF->SBUF DMAs (compute engines can only address partitions
    0/32/64/96; DMA has no such limit). Every pixel matches exactly one
    term.
  - Output returns as bf16 [38.6MB instead of 77MB]; host upcasts to f32.
    Total quantization error is one bf16 rounding: rel err ~4e-3 (< 2e-2).

Self-contained: builds the Bass module, compiles through neuronx_cc via the
bass2jax custom call, and runs SPMD on 8 NeuronCores via shard_map.
"""

import sys

sys.path.insert(0, "/opt/trn_rl_repo")

import numpy as np

_CACHE = {}

_B, _H, _W = 16, 224, 224  # per-core shard
_HALO = 6
_WIN = 2 * _HALO + 1
_G = 4  # images per round


def _build_module(B=_B, H=_H, W=_W, G=_G, HALO=_HALO):
    from concourse import mybir, bacc
    import concourse.tile as tile

    BF16 = mybir.dt.bfloat16
    F16 = mybir.dt.float16
    I16 = mybir.dt.int16
    U8 = mybir.dt.uint8
    Alu = mybir.AluOpType

    NB = 2               # row blocks per image
    RB = H // NB         # 112 output rows per block
    WPAD = W + 2 * HALO  # 236 x-padded plane width
    WIN = 2 * HALO + 1
    R3 = W * 3
    # Band partition layout: 0..RB-1 = central rows r0..r0+RB-1,
    # RB..RB+HALO-1 = top halo rows r0-HALO..r0-1 (mod H),
    # RB+HALO..RB+2*HALO-1 = bottom halo rows r0+RB..+RB+HALO-1 (mod H).
    PT = RB
    PB = RB + HALO
    BAND = RB + 2 * HALO

    nc = bacc.Bacc(None, target_bir_lowering=False)
    img = nc.declare_dram_parameter("img", [B * H, R3], BF16, isOutput=False)
    code = nc.declare_dram_parameter("code", [B * H, W], U8, isOutput=False)
    y = nc.declare_dram_parameter("y", [B * H, R3], BF16, isOutput=True)

    with tile.TileContext(nc) as tc:
        with (
            tc.tile_pool(name="rec", bufs=2) as rpool,
            tc.tile_pool(name="planes", bufs=2) as ppool,
            tc.tile_pool(name="shift", bufs=2) as spool,
            tc.tile_pool(name="tmp", bufs=2) as tpool,
            tc.tile_pool(name="outp", bufs=2) as opool,
        ):
            for g0 in range(0, B, G):
                for blk in range(NB):
                    r0 = blk * RB
                    rtop = (r0 - HALO) % H
                    rbot = (r0 + RB) % H
                    band = rpool.tile([128, G * R3], BF16, tag="band")
                    codet = tpool.tile([128, G * W], U8, tag="codet")
                    for gi in range(G):
                        b = g0 + gi
                        fs = slice(gi * R3, (gi + 1) * R3)
                        nc.sync.dma_start(
                            out=band[0:RB, fs],
                            in_=img[b * H + r0 : b * H + r0 + RB, :],
                        )
                        nc.sync.dma_start(
                            out=band[PT : PT + HALO, fs],
                            in_=img[b * H + rtop : b * H + rtop + HALO, :],
                        )
                        nc.sync.dma_start(
                            out=band[PB : PB + HALO, fs],
                            in_=img[b * H + rbot : b * H + rbot + HALO, :],
                        )
                        nc.sync.dma_start(
                            out=codet[0:RB, gi * W : (gi + 1) * W],
                            in_=code[b * H + r0 : b * H + r0 + RB, :],
                        )

                    band4 = band[:].rearrange("p (g w c) -> p g w c", g=G, c=3)

                    # x-wrap-padded bf16 channel planes [BAND, G, WPAD]
                    planes = []
                    for c in range(3):
                        pc = ppool.tile([128, G * WPAD], BF16, tag=f"plane{c}")
                        pc3 = pc[:].rearrange("p (g w) -> p g w", g=G)
                        nc.vector.tensor_copy(
                            out=pc3[0:BAND, :, HALO : HALO + W],
                            in_=band4[0:BAND, :, :, c : c + 1].rearrange(
                                "p g w k -> p g (w k)"
                            ),
                        )
                        nc.vector.tensor_copy(
                            out=pc3[0:BAND, :, 0:HALO],
                            in_=band4[
                                0:BAND, :, W - HALO : W, c : c + 1
                            ].rearrange("p g w k -> p g (w k)"),
                        )
                        nc.vector.tensor_copy(
                            out=pc3[0:BAND, :, HALO + W : WPAD],
                            in_=band4[0:BAND, :, 0:HALO, c : c + 1].rearrange(
                                "p g w k -> p g (w k)"
                            ),
                        )
                        planes.append(pc)

                    def s(t):
                        return t[0:RB, :]

                    # code as fp16 so the 169 is_equal passes run in the
                    # DVE 16-bit fast mode
                    codeh = tpool.tile([128, G * W], F16, tag="codeh")
                    nc.vector.tensor_copy(out=s(codeh), in_=s(codet))

                    # Pad per-image strides of mask/out so their interp views
                    # cannot dim-merge (copy_predicated needs all three
                    # operand views shaped identically (RB, G, W)).
                    OSTR = R3 + 16
                    MSTR = W + 16
                    out_t = opool.tile([128, G * OSTR], BF16, tag="out")
                    out4 = (
                        out_t[:]
                        .rearrange("p (g q) -> p g q", g=G)[:, :, 0:R3]
                        .rearrange("p g (w c) -> p g w c", c=3)
                    )
                    maskh = tpool.tile([128, G * MSTR], I16, tag="maskh")
                    mview = maskh[:].rearrange("p (g q) -> p g q", g=G)[
                        0:RB, :, 0:W
                    ]
                    ch3 = codeh[:].rearrange("p (g w) -> p g w", g=G)

                    for oy in range(-HALO, HALO + 1):
                        # partition-shifted plane copies (DMA may start at
                        # any partition; compute engines may not). Band
                        # layout makes each shift at most 2 contiguous
                        # pieces.
                        if oy == 0:
                            sps = planes
                        else:
                            sps = []
                            for c in range(3):
                                sp = spool.tile(
                                    [128, G * WPAD], BF16, tag=f"sp{c}"
                                )
                                if oy < 0:
                                    k = -oy
                                    nc.sync.dma_start(
                                        out=sp[0:k, :],
                                        in_=planes[c][PB - k : PB, :],
                                    )
                                    nc.sync.dma_start(
                                        out=sp[k:RB, :],
                                        in_=planes[c][0 : RB - k, :],
                                    )
                                else:
                                    nc.sync.dma_start(
                                        out=sp[0 : RB - oy, :],
                                        in_=planes[c][oy:RB, :],
                                    )
                                    nc.sync.dma_start(
                                        out=sp[RB - oy : RB, :],
                                        in_=planes[c][PB : PB + oy, :],
                                    )
                                sps.append(sp)
                        spv = [
                            t[:].rearrange("p (g w) -> p g w", g=G) for t in sps
                        ]
                        for ox in range(-HALO, HALO + 1):
                            t_code = float((oy + HALO) * WIN + (ox + HALO))
                            nc.vector.tensor_scalar(
                                out=mview, in0=ch3[0:RB],
                                scalar1=t_code, scalar2=None,
                                op0=Alu.is_equal,
                            )
                            for c in range(3):
                                nc.vector.copy_predicated(
                                    out4[0:RB, :, :, c : c + 1].rearrange(
                                        "p g w k -> p g (w k)"
                                    ),
                                    mview,
                                    spv[c][0:RB, :, HALO + ox : HALO + ox + W],
                                )

                    for gi in range(G):
                        b = g0 + gi
                        nc.sync.dma_start(
                            out=y[b * H + r0 : b * H + r0 + RB, :],
                            in_=out_t[0:RB, gi * OSTR : gi * OSTR + R3],
                        )
    return nc


def _split_multiwait_drains(nc):
    """This walrus build accepts one sync wait per Drain (TPB_CTRL); split
    the Tile epilogue's multi-wait drains into single-wait chains."""
    import copy
    import bass_rust
    from concourse import mybir

    changed = False
    new_functions = []
    for function in nc.m.functions:
        new_function = copy.replace(function, blocks=[])
        new_function.set_allocations_from_list(function.allocations)
        for block in function.blocks:
            new_insts = []
            for ins in block.instructions:
                si = ins.sync_info
                if (
                    isinstance(ins, (mybir.InstDrain, mybir.InstNoOp))
                    and si is not None
                    and len(si.on_wait) > 1
                ):
                    changed = True
                    waits = list(si.on_wait)
                    for i, w in enumerate(waits[:-1]):
                        d = mybir.InstDrain(
                            name=f"{ins.name}_sw{i}", ins=[], outs=[],
                            bass_is_fusable=False,
                        )
                        d.engine = ins.engine
                        d.sync_info = bass_rust.SyncInfo(on_wait=[w], on_update=[])
                        new_insts.append(d)
                    ins.sync_info = bass_rust.SyncInfo(
                        on_wait=[waits[-1]], on_update=list(si.on_update)
                    )
                new_insts.append(ins)
            new_function.blocks.append(copy.replace(block, instructions=new_insts))
        new_functions.append(new_function)
    if changed:
        nc.m = copy.replace(nc.m, functions=new_functions)
    return nc


class _Runner:
    def __init__(self, nc, n_cores=8):
        import jax
        from jax.sharding import Mesh, PartitionSpec, NamedSharding
        from jax.experimental.shard_map import shard_map
        from concourse import mybir
        from concourse.bass2jax import (
            _bass_exec_p,
            install_neuronx_cc_hook,
            partition_id_tensor,
        )

        install_neuronx_cc_hook()
        if not nc.is_finalized():
            nc.finalize()
        _split_multiwait_drains(nc)

        self.jax = jax
        partition_name = (
            nc.partition_id_tensor.name if nc.partition_id_tensor else None
        )
        in_names, out_names, out_avals, zero_shapes = [], [], [], []
        for alloc in nc.m.functions[0].allocations:
            if not isinstance(alloc, mybir.MemoryLocationSet):
                continue
            name = alloc.memorylocations[0].name
            if alloc.kind == "ExternalInput":
                if name != partition_name:
                    in_names.append(name)
            elif alloc.kind == "ExternalOutput":
                out_names.append(name)
                shape = tuple(alloc.tensor_shape)
                dtype = mybir.dt.np(alloc.dtype)
                out_avals.append(jax.core.ShapedArray(shape, dtype))
                zero_shapes.append((shape, dtype))
        n_params = len(in_names)
        n_outs = len(out_avals)
        all_in_names = list(in_names) + list(out_names)
        if partition_name is not None:
            all_in_names.append(partition_name)
        donate = tuple(range(n_params, n_params + n_outs))

        def _body(*args):
            operands = list(args)
            if partition_name is not None:
                operands.append(partition_id_tensor())
            outs = _bass_exec_p.bind(
                *operands,
                out_avals=tuple(out_avals),
                in_names=tuple(all_in_names),
                out_names=tuple(out_names),
                lowering_input_output_aliases=(),
                sim_require_finite=True,
                sim_require_nnan=True,
                nc=nc,
            )
            return tuple(outs)

        devices = jax.devices()[:n_cores]
        mesh = Mesh(np.asarray(devices), ("core",))
        in_specs = (PartitionSpec("core"),) * (n_params + n_outs)
        out_specs = (PartitionSpec("core"),) * n_outs
        self.sharded = jax.jit(
            shard_map(
                _body, mesh=mesh, in_specs=in_specs, out_specs=out_specs,
                check_rep=False,
            ),
            donate_argnums=donate,
            keep_unused=True,
        )
        self.devices = devices
        self.shard = NamedSharding(mesh, PartitionSpec("core"))
        self.in_names, self.out_names = in_names, out_names
        self.out_avals, self.zero_shapes = out_avals, zero_shapes
        self.n_cores = n_cores
        # y-init buffers: uploaded once, then recycled from the previous
        # call's outputs (the kernel fully overwrites y; donation consumes
        # the buffers each call)
        self._ybufs = None

    def prep_inputs(self, in_maps):
        """Upload per-core shards in parallel (one device_put per device)."""
        from concurrent.futures import ThreadPoolExecutor

        jax = self.jax
        arrays = []
        for name in self.in_names:
            shards = [np.asarray(m[name]) for m in in_maps]
            full_shape = (
                self.n_cores * shards[0].shape[0],
                *shards[0].shape[1:],
            )
            with ThreadPoolExecutor(self.n_cores) as ex:
                parts = list(
                    ex.map(
                        lambda t: jax.device_put(t[0], t[1]),
                        zip(shards, self.devices),
                    )
                )
            arrays.append(
                jax.make_array_from_single_device_arrays(
                    full_shape, self.shard, parts
                )
            )
        jax.block_until_ready(arrays)
        return arrays

    def _get_ybufs(self):
        if self._ybufs is None:
            jax = self.jax
            zs = [
                jax.device_put(
                    np.zeros((self.n_cores * s[0], *s[1:]), d), self.shard
                )
                for (s, d) in self.zero_shapes
            ]
            jax.block_until_ready(zs)
            self._ybufs = zs
        return self._ybufs

    def run(self, dev_in):
        out = self.sharded(*dev_in, *self._get_ybufs())
        self.jax.block_until_ready(out)
        self._ybufs = list(out)
        return out

    def fetch(self, out):
        """Download the output shards in parallel."""
        from concurrent.futures import ThreadPoolExecutor

        res = []
        for arr in out:
            shards = sorted(
                arr.addressable_shards, key=lambda s: s.index[0].start
            )
            with ThreadPoolExecutor(self.n_cores) as ex:
                parts = list(ex.map(lambda s: np.asarray(s.data), shards))
            res.append(parts)
        return res

    def run_maps(self, in_maps):
        parts = self.fetch(self.run(self.prep_inputs(in_maps)))
        return [
            {name: parts[i][c] for i, name in enumerate(self.out_names)}
            for c in range(self.n_cores)
        ]


def _get_runner():
    if "r" not in _CACHE:
        _CACHE["r"] = _Runner(_build_module())
    return _CACHE["r"]


def _host_code(dx, dy):
    """Window-offset code per pixel, bit-exact vs the CPU jax reference.
    Returns (code uint8, ok bool)."""
    H, W = _H, _W
    cols = np.arange(W, dtype=np.float32)[None, None, :]
    rows = np.arange(H, dtype=np.float32)[None, :, None]

    def off(t, lim, base_i):
        # wrap into [0, lim] exactly as f32 mod does, floor via int cast,
        # clamp the t==lim boundary, then window-normalize around base
        tw = np.where(
            t < 0, t + np.float32(lim),
            np.where(t >= lim, t - np.float32(lim), t),
        ).astype(np.int32)
        o = np.minimum(tw, lim - 1) - base_i
        o = np.where(o > 112, o - lim, o)
        return np.where(o < -112, o + lim, o)

    offx = off(cols + dx, W, np.arange(W, dtype=np.int32)[None, None, :])
    offy = off(rows + dy, H, np.arange(H, dtype=np.int32)[None, :, None])
    ok = bool(
        offx.min() >= -_HALO and offx.max() <= _HALO
        and offy.min() >= -_HALO and offy.max() <= _HALO
    )
    if not ok:
        return None, False
    codes = ((offy + _HALO) * _WIN + (offx + _HALO)).astype(np.uint8)
    return codes, True


def _kernel_np(x):
    """Exact reference semantics (including jax's clamp of the f32 mod
    boundary case) — robustness fallback."""
    H, W = _H, _W
    img = x[..., 0:3]
    dx = x[..., 3]
    dy = x[..., 4]
    cols = np.arange(W, dtype=np.float32)
    rows = np.arange(H, dtype=np.float32)[:, None]
    Xi = np.minimum(
        np.mod(cols[None, None, :] + dx, np.float32(W)).astype(np.int32), W - 1
    )
    Yi = np.minimum(
        np.mod(rows[None, :, :] + dy, np.float32(H)).astype(np.int32), H - 1
    )
    b = np.arange(x.shape[0])[:, None, None]
    return img[b, Yi, Xi]


def _kernel_jax_device(x):
    """Tier-2 fallback: run the warp gather on the 8 NeuronCores via
    XLA-Neuron's native gather path."""
    import jax
    import jax.numpy as jnp

    H, W = _H, _W

    def body(xs):  # [B, H, W, 5] per device
        img = xs[..., 0:3]
        dx = xs[..., 3]
        dy = xs[..., 4]
        cols = jnp.arange(W, dtype=jnp.float32)
        rows = jnp.arange(H, dtype=jnp.float32)[:, None]
        Xi = jnp.mod(cols[None, None, :] + dx, float(W)).astype(jnp.int32)
        Yi = jnp.mod(rows[None, :, :] + dy, float(H)).astype(jnp.int32)
        b = jnp.arange(xs.shape[0])[:, None, None]
        return img[b, Yi, Xi]

    if "jdk" not in _CACHE:
        _CACHE["jdk"] = jax.jit(body)
    f = _CACHE["jdk"]
    devices = jax.devices()[:8]
    shards = x.reshape(8, _B, H, W, 5)
    dev_in = [jax.device_put(shards[i], devices[i]) for i in range(8)]
    outs = [f(s) for s in dev_in]
    host = jax.device_get(outs)
    return np.concatenate(host, axis=0)


_USE_BASS = True


def _kernel_bass(x):
    """Optimized flow: start the (async) img upload first, compute the
    code on the host while it streams, then upload the code, run, and
    upcast during the parallel fetch."""
    from concurrent.futures import ThreadPoolExecutor
    import ml_dtypes
    import jax

    n_cores = 8
    r = _get_runner()

    img16 = np.ascontiguousarray(x[..., 0:3]).astype(ml_dtypes.bfloat16)
    img_sh = img16.reshape(n_cores, _B * _H, _W * 3)
    # async: transfers proceed while we compute the code below
    img_parts = [
        jax.device_put(img_sh[c], r.devices[c]) for c in range(n_cores)
    ]

    codes, ok = _host_code(x[..., 3], x[..., 4])
    if not ok:
        raise ValueError("displacement exceeds the 13x13 window")
    code_sh = codes.reshape(n_cores, _B * _H, _W)
    code_parts = [
        jax.device_put(code_sh[c], r.devices[c]) for c in range(n_cores)
    ]

    arrays = []
    for name, parts, shp in (
        ("img", img_parts, (n_cores * _B * _H, _W * 3)),
        ("code", code_parts, (n_cores * _B * _H, _W)),
    ):
        arrays.append(
            jax.make_array_from_single_device_arrays(shp, r.shard, parts)
        )
    assert r.in_names == ["img", "code"], r.in_names
    jax.block_until_ready(arrays)

    out = r.run(arrays)[0]
    shards = sorted(out.addressable_shards, key=lambda s: s.index[0].start)
    y = np.empty((128, _H, _W, 3), np.float32)
    yv = y.reshape(n_cores, _B * _H, _W * 3)

    def fetch_one(ci):
        c, s = ci
        yv[c] = np.asarray(s.data).astype(np.float32)

    with ThreadPoolExecutor(n_cores) as ex:
        list(ex.map(fetch_one, enumerate(shards)))
    return y


def kernel(x):
    x = np.ascontiguousarray(np.asarray(x, dtype=np.float32))
    assert x.shape == (128, _H, _W, 5), x.shape
    if _USE_BASS:
        try:
            return _kernel_bass(x)
        except Exception as e:
            sys.stderr.write(
                f"kernel: bass path failed ({e!r}); jax-device fallback\n"
            )
    try:
        return _kernel_jax_device(x)
    except Exception as e:
        sys.stderr.write(f"kernel: jax-device failed ({e!r}); numpy fallback\n")
        return _kernel_np(x)


# revision 20
# speedup vs baseline: 1.3760x; 1.1120x over previous
"""Trainium2 Bass kernel for nn_Bilinear_70222715290053.

Problem: x [128, 224, 224, 5] f32 where channels 0:3 are an image and
channels 3,4 are per-pixel displacements (dx, dy). Output [128,224,224,3]:
  out[b,i,j,:] = img[b, int(mod(i+dy, 224)), int(mod(j+dx, 224)), :]

Key property: dx, dy ~ N(0,1), so |displacement| <= ~5.5 — the gather is a
LOCAL warp within a 13x13 window (kernel() verifies the bound at runtime
and falls back to an exact path if violated).

System design (the host<->device axon tunnel runs at ~40MB/s and is the
end-to-end bottleneck, so minimize bytes moved):
  - Host side (cheap, vectorized, bit-exact vs the CPU jax reference —
    verified): compute the source-pixel window offsets and fold them into
    one uint8 code = (offy+6)*13 + (offx+6) per pixel [6.4MB], and convert
    the 3 image channels to bf16 [38.6MB]. Upload 45MB instead of 128MB.
  - Device side (Bass, SPMD on 8 cores, batch-sharded 16 images/core):
    rows live in the partition dim; a round processes G=4 images x 112
    output rows (+6 halo rows each side via extra partitions, mod-224
    wrapped). 169 select terms: mask = is_equal(code, t) on the DVE, then
    3 copy_predicated moves from the (oy, ox)-shifted x-padded channel
    planes into the output tile. Partition shifts are materialized by
    cheap SBUF->SBUF DMAs (compute engines can only address partitions
    0/32/64/96; DMA has no such limit). Every pixel matches exactly one
    term.
  - Output returns as bf16 [38.6MB instead of 77MB]; host upcasts to f32.
    Total quantization error is one bf16 rounding: rel err ~4e-3 (< 2e-2).

Self-contained: builds the Bass module, compiles through neuronx_cc via the
bass2jax custom call, and runs SPMD on 8 NeuronCores via shard_map.
"""

import sys

sys.path.insert(0, "/opt/trn_rl_repo")

import numpy as np

_CACHE = {}

_B, _H, _W = 16, 224, 224  # per-core shard
_HALO = 6
_WIN = 2 * _HALO + 1
_G = 4  # images per round


def _build_module(B=_B, H=_H, W=_W, G=_G, HALO=_HALO):
    from concourse import mybir, bacc
    import concourse.tile as tile

    BF16 = mybir.dt.bfloat16
    F16 = mybir.dt.float16
    I16 = mybir.dt.int16
    U8 = mybir.dt.uint8
    Alu = mybir.AluOpType

    NB = 2               # row blocks per image
    RB = H // NB         # 112 output rows per block
    WPAD = W + 2 * HALO  # 236 x-padded plane width
    WIN = 2 * HALO + 1
    R3 = W * 3
    # Band partition layout: 0..RB-1 = central rows r0..r0+RB-1,
    # RB..RB+HALO-1 = top halo rows r0-HALO..r0-1 (mod H),
    # RB+HALO..RB+2*HALO-1 = bottom halo rows r0+RB..+RB+HALO-1 (mod H).
    PT = RB
    PB = RB + HALO
    BAND = RB + 2 * HALO

    nc = bacc.Bacc(None, target_bir_lowering=False)
    img = nc.declare_dram_parameter("img", [B * H, R3], BF16, isOutput=False)
    code = nc.declare_dram_parameter("code", [B * H, W], U8, isOutput=False)
    y = nc.declare_dram_parameter("y", [B * H, R3], BF16, isOutput=True)

    with tile.TileContext(nc) as tc:
        with (
            tc.tile_pool(name="rec", bufs=2) as rpool,
            tc.tile_pool(name="planes", bufs=2) as ppool,
            tc.tile_pool(name="shift", bufs=2) as spool,
            tc.tile_pool(name="tmp", bufs=2) as tpool,
            tc.tile_pool(name="outp", bufs=2) as opool,
        ):
            for g0 in range(0, B, G):
                for blk in range(NB):
                    r0 = blk * RB
                    rtop = (r0 - HALO) % H
                    rbot = (r0 + RB) % H
                    band = rpool.tile([128, G * R3], BF16, tag="band")
                    codet = tpool.tile([128, G * W], U8, tag="codet")
                    for gi in range(G):
                        b = g0 + gi
                        fs = slice(gi * R3, (gi + 1) * R3)
                        nc.sync.dma_start(
                            out=band[0:RB, fs],
                            in_=img[b * H + r0 : b * H + r0 + RB, :],
                        )
                        nc.sync.dma_start(
                            out=band[PT : PT + HALO, fs],
                            in_=img[b * H + rtop : b * H + rtop + HALO, :],
                        )
                        nc.sync.dma_start(
                            out=band[PB : PB + HALO, fs],
                            in_=img[b * H + rbot : b * H + rbot + HALO, :],
                        )
                        nc.sync.dma_start(
                            out=codet[0:RB, gi * W : (gi + 1) * W],
                            in_=code[b * H + r0 : b * H + r0 + RB, :],
                        )

                    band4 = band[:].rearrange("p (g w c) -> p g w c", g=G, c=3)

                    # x-wrap-padded bf16 channel planes [BAND, G, WPAD]
                    planes = []
                    for c in range(3):
                        pc = ppool.tile([128, G * WPAD], BF16, tag=f"plane{c}")
                        pc3 = pc[:].rearrange("p (g w) -> p g w", g=G)
                        nc.vector.tensor_copy(
                            out=pc3[0:BAND, :, HALO : HALO + W],
                            in_=band4[0:BAND, :, :, c : c + 1].rearrange(
                                "p g w k -> p g (w k)"
                            ),
                        )
                        nc.vector.tensor_copy(
                            out=pc3[0:BAND, :, 0:HALO],
                            in_=band4[
                                0:BAND, :, W - HALO : W, c : c + 1
                            ].rearrange("p g w k -> p g (w k)"),
                        )
                        nc.vector.tensor_copy(
                            out=pc3[0:BAND, :, HALO + W : WPAD],
                            in_=band4[0:BAND, :, 0:HALO, c : c + 1].rearrange(
                                "p g w k -> p g (w k)"
                            ),
                        )
                        planes.append(pc)

                    def s(t):
                        return t[0:RB, :]

                    # code as fp16 so the 169 is_equal passes run in the
                    # DVE 16-bit fast mode
                    codeh = tpool.tile([128, G * W], F16, tag="codeh")
                    nc.vector.tensor_copy(out=s(codeh), in_=s(codet))

                    # Pad per-image strides of mask/out so their interp views
                    # cannot dim-merge (copy_predicated needs all three
                    # operand views shaped identically (RB, G, W)).
                    OSTR = R3 + 16
                    MSTR = W + 16
                    out_t = opool.tile([128, G * OSTR], BF16, tag="out")
                    out4 = (
                        out_t[:]
                        .rearrange("p (g q) -> p g q", g=G)[:, :, 0:R3]
                        .rearrange("p g (w c) -> p g w c", c=3)
                    )
                    maskh = tpool.tile([128, G * MSTR], I16, tag="maskh")
                    mview = maskh[:].rearrange("p (g q) -> p g q", g=G)[
                        0:RB, :, 0:W
                    ]
                    ch3 = codeh[:].rearrange("p (g w) -> p g w", g=G)

                    for oy in range(-HALO, HALO + 1):
                        # partition-shifted plane copies (DMA may start at
                        # any partition; compute engines may not). Band
                        # layout makes each shift at most 2 contiguous
                        # pieces.
                        if oy == 0:
                            sps = planes
                        else:
                            sps = []
                            for c in range(3):
                                sp = spool.tile(
                                    [128, G * WPAD], BF16, tag=f"sp{c}"
                                )
                                if oy < 0:
                                    k = -oy
                                    nc.sync.dma_start(
                                        out=sp[0:k, :],
                                        in_=planes[c][PB - k : PB, :],
                                    )
                                    nc.sync.dma_start(
                                        out=sp[k:RB, :],
                                        in_=planes[c][0 : RB - k, :],
                                    )
                                else:
                                    nc.sync.dma_start(
                                        out=sp[0 : RB - oy, :],
                                        in_=planes[c][oy:RB, :],
                                    )
                                    nc.sync.dma_start(
                                        out=sp[RB - oy : RB, :],
                                        in_=planes[c][PB : PB + oy, :],
                                    )
                                sps.append(sp)
                        spv = [
                            t[:].rearrange("p (g w) -> p g w", g=G) for t in sps
                        ]
                        for ox in range(-HALO, HALO + 1):
                            t_code = float((oy + HALO) * WIN + (ox + HALO))
                            nc.vector.tensor_scalar(
                                out=mview, in0=ch3[0:RB],
                                scalar1=t_code, scalar2=None,
                                op0=Alu.is_equal,
                            )
                            for c in range(3):
                                nc.vector.copy_predicated(
                                    out4[0:RB, :, :, c : c + 1].rearrange(
                                        "p g w k -> p g (w k)"
                                    ),
                                    mview,
                                    spv[c][0:RB, :, HALO + ox : HALO + ox + W],
                                )

                    for gi in range(G):
                        b = g0 + gi
                        nc.sync.dma_start(
                            out=y[b * H + r0 : b * H + r0 + RB, :],
                            in_=out_t[0:RB, gi * OSTR : gi * OSTR + R3],
                        )
    return nc


def _split_multiwait_drains(nc):
    """This walrus build accepts one sync wait per Drain (TPB_CTRL); split
    the Tile epilogue's multi-wait drains into single-wait chains."""
    import copy
    import bass_rust
    from concourse import mybir

    changed = False
    new_functions = []
    for function in nc.m.functions:
        new_function = copy.replace(function, blocks=[])
        new_function.set_allocations_from_list(function.allocations)
        for block in function.blocks:
            new_insts = []
            for ins in block.instructions:
                si = ins.sync_info
                if (
                    isinstance(ins, (mybir.InstDrain, mybir.InstNoOp))
                    and si is not None
                    and len(si.on_wait) > 1
                ):
                    changed = True
                    waits = list(si.on_wait)
                    for i, w in enumerate(waits[:-1]):
                        d = mybir.InstDrain(
                            name=f"{ins.name}_sw{i}", ins=[], outs=[],
                            bass_is_fusable=False,
                        )
                        d.engine = ins.engine
                        d.sync_info = bass_rust.SyncInfo(on_wait=[w], on_update=[])
                        new_insts.append(d)
                    ins.sync_info = bass_rust.SyncInfo(
                        on_wait=[waits[-1]], on_update=list(si.on_update)
                    )
                new_insts.append(ins)
            new_function.blocks.append(copy.replace(block, instructions=new_insts))
        new_functions.append(new_function)
    if changed:
        nc.m = copy.replace(nc.m, functions=new_functions)
    return nc


class _Runner:
    def __init__(self, nc, n_cores=8):
        import jax
        from jax.sharding import Mesh, PartitionSpec, NamedSharding
        from jax.experimental.shard_map import shard_map
        from concourse import mybir
        from concourse.bass2jax import (
            _bass_exec_p,
            install_neuronx_cc_hook,
            partition_id_tensor,
        )

        install_neuronx_cc_hook()
        if not nc.is_finalized():
            nc.finalize()
        _split_multiwait_drains(nc)

        self.jax = jax
        partition_name = (
            nc.partition_id_tensor.name if nc.partition_id_tensor else None
        )
        in_names, out_names, out_avals, zero_shapes = [], [], [], []
        for alloc in nc.m.functions[0].allocations:
            if not isinstance(alloc, mybir.MemoryLocationSet):
                continue
            name = alloc.memorylocations[0].name
            if alloc.kind == "ExternalInput":
                if name != partition_name:
                    in_names.append(name)
            elif alloc.kind == "ExternalOutput":
                out_names.append(name)
                shape = tuple(alloc.tensor_shape)
                dtype = mybir.dt.np(alloc.dtype)
                out_avals.append(jax.core.ShapedArray(shape, dtype))
                zero_shapes.append((shape, dtype))
        n_params = len(in_names)
        n_outs = len(out_avals)
        all_in_names = list(in_names) + list(out_names)
        if partition_name is not None:
            all_in_names.append(partition_name)
        donate = tuple(range(n_params, n_params + n_outs))

        def _body(*args):
            operands = list(args)
            if partition_name is not None:
                operands.append(partition_id_tensor())
            outs = _bass_exec_p.bind(
                *operands,
                out_avals=tuple(out_avals),
                in_names=tuple(all_in_names),
                out_names=tuple(out_names),
                lowering_input_output_aliases=(),
                sim_require_finite=True,
                sim_require_nnan=True,
                nc=nc,
            )
            return tuple(outs)

        devices = jax.devices()[:n_cores]
        mesh = Mesh(np.asarray(devices), ("core",))
        in_specs = (PartitionSpec("core"),) * (n_params + n_outs)
        out_specs = (PartitionSpec("core"),) * n_outs
        self.sharded = jax.jit(
            shard_map(
                _body, mesh=mesh, in_specs=in_specs, out_specs=out_specs,
                check_rep=False,
            ),
            donate_argnums=donate,
            keep_unused=True,
        )
        self.devices = devices
        self.shard = NamedSharding(mesh, PartitionSpec("core"))
        self.in_names, self.out_names = in_names, out_names
        self.out_avals, self.zero_shapes = out_avals, zero_shapes
        self.n_cores = n_cores
        # y-init buffers: uploaded once, then recycled from the previous
        # call's outputs (the kernel fully overwrites y; donation consumes
        # the buffers each call)
        self._ybufs = None

    def prep_inputs(self, in_maps):
        """Upload per-core shards in parallel (one device_put per device)."""
        from concurrent.futures import ThreadPoolExecutor

        jax = self.jax
        arrays = []
        for name in self.in_names:
            shards = [np.asarray(m[name]) for m in in_maps]
            full_shape = (
                self.n_cores * shards[0].shape[0],
                *shards[0].shape[1:],
            )
            with ThreadPoolExecutor(self.n_cores) as ex:
                parts = list(
                    ex.map(
                        lambda t: jax.device_put(t[0], t[1]),
                        zip(shards, self.devices),
                    )
                )
            arrays.append(
                jax.make_array_from_single_device_arrays(
                    full_shape, self.shard, parts
                )
            )
        jax.block_until_ready(arrays)
        return arrays

    def _get_ybufs(self):
        if self._ybufs is None:
            jax = self.jax
            zs = [
                jax.device_put(
                    np.zeros((self.n_cores * s[0], *s[1:]), d), self.shard
                )
                for (s, d) in self.zero_shapes
            ]
            jax.block_until_ready(zs)
            self._ybufs = zs
        return self._ybufs

    def run(self, dev_in):
        out = self.sharded(*dev_in, *self._get_ybufs())
        self.jax.block_until_ready(out)
        self._ybufs = list(out)
        return out

    def fetch(self, out):
        """Download the output shards in parallel."""
        from concurrent.futures import ThreadPoolExecutor

        res = []
        for arr in out:
            shards = sorted(
                arr.addressable_shards, key=lambda s: s.index[0].start
            )
            with ThreadPoolExecutor(self.n_cores) as ex:
                parts = list(ex.map(lambda s: np.asarray(s.data), shards))
            res.append(parts)
        return res

    def run_maps(self, in_maps):
        parts = self.fetch(self.run(self.prep_inputs(in_maps)))
        return [
            {name: parts[i][c] for i, name in enumerate(self.out_names)}
            for c in range(self.n_cores)
        ]


def _get_runner():
    if "r" not in _CACHE:
        _CACHE["r"] = _Runner(_build_module())
    return _CACHE["r"]


def _host_code(dx, dy):
    """Window-offset code per pixel, bit-exact vs the CPU jax reference.
    Returns (code uint8, ok bool)."""
    H, W = _H, _W
    cols = np.arange(W, dtype=np.float32)[None, None, :]
    rows = np.arange(H, dtype=np.float32)[None, :, None]

    def off(t, lim, base_i):
        # wrap into [0, lim] exactly as f32 mod does, floor via int cast,
        # clamp the t==lim boundary, then window-normalize around base
        tw = np.where(
            t < 0, t + np.float32(lim),
            np.where(t >= lim, t - np.float32(lim), t),
        ).astype(np.int32)
        o = np.minimum(tw, lim - 1) - base_i
        o = np.where(o > 112, o - lim, o)
        return np.where(o < -112, o + lim, o)

    offx = off(cols + dx, W, np.arange(W, dtype=np.int32)[None, None, :])
    offy = off(rows + dy, H, np.arange(H, dtype=np.int32)[None, :, None])
    ok = bool(
        offx.min() >= -_HALO and offx.max() <= _HALO
        and offy.min() >= -_HALO and offy.max() <= _HALO
    )
    if not ok:
        return None, False
    codes = ((offy + _HALO) * _WIN + (offx + _HALO)).astype(np.uint8)
    return codes, True


def _kernel_np(x):
    """Exact reference semantics (including jax's clamp of the f32 mod
    boundary case) — robustness fallback."""
    H, W = _H, _W
    img = x[..., 0:3]
    dx = x[..., 3]
    dy = x[..., 4]
    cols = np.arange(W, dtype=np.float32)
    rows = np.arange(H, dtype=np.float32)[:, None]
    Xi = np.minimum(
        np.mod(cols[None, None, :] + dx, np.float32(W)).astype(np.int32), W - 1
    )
    Yi = np.minimum(
        np.mod(rows[None, :, :] + dy, np.float32(H)).astype(np.int32), H - 1
    )
    b = np.arange(x.shape[0])[:, None, None]
    return img[b, Yi, Xi]


def _kernel_jax_device(x):
    """Tier-2 fallback: run the warp gather on the 8 NeuronCores via
    XLA-Neuron's native gather path."""
    import jax
    import jax.numpy as jnp

    H, W = _H, _W

    def body(xs):  # [B, H, W, 5] per device
        img = xs[..., 0:3]
        dx = xs[..., 3]
        dy = xs[..., 4]
        cols = jnp.arange(W, dtype=jnp.float32)
        rows = jnp.arange(H, dtype=jnp.float32)[:, None]
        Xi = jnp.mod(cols[None, None, :] + dx, float(W)).astype(jnp.int32)
        Yi = jnp.mod(rows[None, :, :] + dy, float(H)).astype(jnp.int32)
        b = jnp.arange(xs.shape[0])[:, None, None]
        return img[b, Yi, Xi]

    if "jdk" not in _CACHE:
        _CACHE["jdk"] = jax.jit(body)
    f = _CACHE["jdk"]
    devices = jax.devices()[:8]
    shards = x.reshape(8, _B, H, W, 5)
    dev_in = [jax.device_put(shards[i], devices[i]) for i in range(8)]
    outs = [f(s) for s in dev_in]
    host = jax.device_get(outs)
    return np.concatenate(host, axis=0)


_USE_BASS = True


def _kernel_bass(x):
    """Optimized flow: start the (async) img upload first, compute the
    code on the host while it streams, then upload the code, run, and
    upcast during the parallel fetch."""
    from concurrent.futures import ThreadPoolExecutor
    import ml_dtypes
    import jax

    from concurrent.futures import ThreadPoolExecutor as _TPE

    n_cores = 8
    r = _get_runner()

    # pipeline: each shard is converted to bf16 in a worker thread and its
    # (async) upload issued immediately; the window code is computed on the
    # main thread while the uploads stream
    x8 = x.reshape(n_cores, _B, _H, _W, 5)

    def conv_put(c):
        a = np.ascontiguousarray(x8[c, ..., 0:3]).astype(ml_dtypes.bfloat16)
        return jax.device_put(a.reshape(_B * _H, _W * 3), r.devices[c])

    pool = _TPE(n_cores)
    img_futs = [pool.submit(conv_put, c) for c in range(n_cores)]

    codes, ok = _host_code(x[..., 3], x[..., 4])
    if not ok:
        pool.shutdown(wait=True)
        raise ValueError("displacement exceeds the 13x13 window")
    code_sh = codes.reshape(n_cores, _B * _H, _W)
    code_parts = [
        jax.device_put(code_sh[c], r.devices[c]) for c in range(n_cores)
    ]
    img_parts = [f.result() for f in img_futs]
    pool.shutdown(wait=False)

    arrays = []
    for name, parts, shp in (
        ("img", img_parts, (n_cores * _B * _H, _W * 3)),
        ("code", code_parts, (n_cores * _B * _H, _W)),
    ):
        arrays.append(
            jax.make_array_from_single_device_arrays(shp, r.shard, parts)
        )
    assert r.in_names == ["img", "code"], r.in_names
    jax.block_until_ready(arrays)

    out = r.run(arrays)[0]
    shards = sorted(out.addressable_shards, key=lambda s: s.index[0].start)
    y = np.empty((128, _H, _W, 3), np.float32)
    yv = y.reshape(n_cores, _B * _H, _W * 3)

    def fetch_one(ci):
        c, s = ci
        yv[c] = np.asarray(s.data).astype(np.float32)

    with ThreadPoolExecutor(n_cores) as ex:
        list(ex.map(fetch_one, enumerate(shards)))
    return y


def kernel(x):
    x = np.ascontiguousarray(np.asarray(x, dtype=np.float32))
    assert x.shape == (128, _H, _W, 5), x.shape
    if _USE_BASS:
        try:
            return _kernel_bass(x)
        except Exception as e:
            sys.stderr.write(
                f"kernel: bass path failed ({e!r}); jax-device fallback\n"
            )
    try:
        return _kernel_jax_device(x)
    except Exception as e:
        sys.stderr.write(f"kernel: jax-device failed ({e!r}); numpy fallback\n")
        return _kernel_np(x)


# revision 22
# speedup vs baseline: 1.5185x; 1.1036x over previous
"""Trainium2 Bass kernel for nn_Bilinear_70222715290053.

Problem: x [128, 224, 224, 5] f32 where channels 0:3 are an image and
channels 3,4 are per-pixel displacements (dx, dy). Output [128,224,224,3]:
  out[b,i,j,:] = img[b, int(mod(i+dy, 224)), int(mod(j+dx, 224)), :]

Key property: dx, dy ~ N(0,1), so |displacement| <= ~5.5 — the gather is a
LOCAL warp within a 13x13 window (kernel() verifies the bound at runtime
and falls back to an exact path if violated).

System design (the host<->device axon tunnel runs at ~40MB/s and is the
end-to-end bottleneck, so minimize bytes moved):
  - Host side (cheap, vectorized, bit-exact vs the CPU jax reference —
    verified): compute the source-pixel window offsets and fold them into
    one uint8 code = (offy+6)*13 + (offx+6) per pixel [6.4MB], and convert
    the 3 image channels to bf16 [38.6MB]. Upload 45MB instead of 128MB.
  - Device side (Bass, SPMD on 8 cores, batch-sharded 16 images/core):
    rows live in the partition dim; a round processes G=4 images x 112
    output rows (+6 halo rows each side via extra partitions, mod-224
    wrapped). 169 select terms: mask = is_equal(code, t) on the DVE, then
    3 copy_predicated moves from the (oy, ox)-shifted x-padded channel
    planes into the output tile. Partition shifts are materialized by
    cheap SBUF->SBUF DMAs (compute engines can only address partitions
    0/32/64/96; DMA has no such limit). Every pixel matches exactly one
    term.
  - Output returns as bf16 [38.6MB instead of 77MB]; host upcasts to f32.
    Total quantization error is one bf16 rounding: rel err ~4e-3 (< 2e-2).

Self-contained: builds the Bass module, compiles through neuronx_cc via the
bass2jax custom call, and runs SPMD on 8 NeuronCores via shard_map.
"""

import sys

sys.path.insert(0, "/opt/trn_rl_repo")

import numpy as np

_CACHE = {}

_B, _H, _W = 16, 224, 224  # per-core shard
_HALO = 6
_WIN = 2 * _HALO + 1
_G = 4  # images per round


def _build_module(B=_B, H=_H, W=_W, G=_G, HALO=_HALO):
    from concourse import mybir, bacc
    import concourse.tile as tile

    BF16 = mybir.dt.bfloat16
    F16 = mybir.dt.float16
    I16 = mybir.dt.int16
    U8 = mybir.dt.uint8
    Alu = mybir.AluOpType

    NB = 2               # row blocks per image
    RB = H // NB         # 112 output rows per block
    WPAD = W + 2 * HALO  # 236 x-padded plane width
    WIN = 2 * HALO + 1
    R3 = W * 3
    # Band partition layout: 0..RB-1 = central rows r0..r0+RB-1,
    # RB..RB+HALO-1 = top halo rows r0-HALO..r0-1 (mod H),
    # RB+HALO..RB+2*HALO-1 = bottom halo rows r0+RB..+RB+HALO-1 (mod H).
    PT = RB
    PB = RB + HALO
    BAND = RB + 2 * HALO

    nc = bacc.Bacc(None, target_bir_lowering=False)
    img = nc.declare_dram_parameter("img", [B * H, R3], BF16, isOutput=False)
    code = nc.declare_dram_parameter("code", [B * H, W], U8, isOutput=False)
    y = nc.declare_dram_parameter("y", [B * H, R3], BF16, isOutput=True)

    with tile.TileContext(nc) as tc:
        with (
            tc.tile_pool(name="rec", bufs=2) as rpool,
            tc.tile_pool(name="planes", bufs=2) as ppool,
            tc.tile_pool(name="shift", bufs=2) as spool,
            tc.tile_pool(name="tmp", bufs=2) as tpool,
            tc.tile_pool(name="outp", bufs=2) as opool,
        ):
            for g0 in range(0, B, G):
                for blk in range(NB):
                    r0 = blk * RB
                    rtop = (r0 - HALO) % H
                    rbot = (r0 + RB) % H
                    band = rpool.tile([128, G * R3], BF16, tag="band")
                    codet = tpool.tile([128, G * W], U8, tag="codet")
                    for gi in range(G):
                        b = g0 + gi
                        fs = slice(gi * R3, (gi + 1) * R3)
                        nc.sync.dma_start(
                            out=band[0:RB, fs],
                            in_=img[b * H + r0 : b * H + r0 + RB, :],
                        )
                        nc.sync.dma_start(
                            out=band[PT : PT + HALO, fs],
                            in_=img[b * H + rtop : b * H + rtop + HALO, :],
                        )
                        nc.sync.dma_start(
                            out=band[PB : PB + HALO, fs],
                            in_=img[b * H + rbot : b * H + rbot + HALO, :],
                        )
                        nc.sync.dma_start(
                            out=codet[0:RB, gi * W : (gi + 1) * W],
                            in_=code[b * H + r0 : b * H + r0 + RB, :],
                        )

                    band4 = band[:].rearrange("p (g w c) -> p g w c", g=G, c=3)

                    # x-wrap-padded bf16 channel planes [BAND, G, WPAD]
                    planes = []
                    for c in range(3):
                        pc = ppool.tile([128, G * WPAD], BF16, tag=f"plane{c}")
                        pc3 = pc[:].rearrange("p (g w) -> p g w", g=G)
                        nc.vector.tensor_copy(
                            out=pc3[0:BAND, :, HALO : HALO + W],
                            in_=band4[0:BAND, :, :, c : c + 1].rearrange(
                                "p g w k -> p g (w k)"
                            ),
                        )
                        nc.vector.tensor_copy(
                            out=pc3[0:BAND, :, 0:HALO],
                            in_=band4[
                                0:BAND, :, W - HALO : W, c : c + 1
                            ].rearrange("p g w k -> p g (w k)"),
                        )
                        nc.vector.tensor_copy(
                            out=pc3[0:BAND, :, HALO + W : WPAD],
                            in_=band4[0:BAND, :, 0:HALO, c : c + 1].rearrange(
                                "p g w k -> p g (w k)"
                            ),
                        )
                        planes.append(pc)

                    def s(t):
                        return t[0:RB, :]

                    # code as fp16 so the 169 is_equal passes run in the
                    # DVE 16-bit fast mode
                    codeh = tpool.tile([128, G * W], F16, tag="codeh")
                    nc.vector.tensor_copy(out=s(codeh), in_=s(codet))

                    # Pad per-image strides of mask/out so their interp views
                    # cannot dim-merge (copy_predicated needs all three
                    # operand views shaped identically (RB, G, W)).
                    OSTR = R3 + 16
                    MSTR = W + 16
                    out_t = opool.tile([128, G * OSTR], BF16, tag="out")
                    out4 = (
                        out_t[:]
                        .rearrange("p (g q) -> p g q", g=G)[:, :, 0:R3]
                        .rearrange("p g (w c) -> p g w c", c=3)
                    )
                    maskh = tpool.tile([128, G * MSTR], I16, tag="maskh")
                    mview = maskh[:].rearrange("p (g q) -> p g q", g=G)[
                        0:RB, :, 0:W
                    ]
                    ch3 = codeh[:].rearrange("p (g w) -> p g w", g=G)

                    for oy in range(-HALO, HALO + 1):
                        # partition-shifted plane copies (DMA may start at
                        # any partition; compute engines may not). Band
                        # layout makes each shift at most 2 contiguous
                        # pieces.
                        if oy == 0:
                            sps = planes
                        else:
                            sps = []
                            for c in range(3):
                                sp = spool.tile(
                                    [128, G * WPAD], BF16, tag=f"sp{c}"
                                )
                                if oy < 0:
                                    k = -oy
                                    nc.sync.dma_start(
                                        out=sp[0:k, :],
                                        in_=planes[c][PB - k : PB, :],
                                    )
                                    nc.sync.dma_start(
                                        out=sp[k:RB, :],
                                        in_=planes[c][0 : RB - k, :],
                                    )
                                else:
                                    nc.sync.dma_start(
                                        out=sp[0 : RB - oy, :],
                                        in_=planes[c][oy:RB, :],
                                    )
                                    nc.sync.dma_start(
                                        out=sp[RB - oy : RB, :],
                                        in_=planes[c][PB : PB + oy, :],
                                    )
                                sps.append(sp)
                        spv = [
                            t[:].rearrange("p (g w) -> p g w", g=G) for t in sps
                        ]
                        for ox in range(-HALO, HALO + 1):
                            t_code = float((oy + HALO) * WIN + (ox + HALO))
                            nc.vector.tensor_scalar(
                                out=mview, in0=ch3[0:RB],
                                scalar1=t_code, scalar2=None,
                                op0=Alu.is_equal,
                            )
                            for c in range(3):
                                nc.vector.copy_predicated(
                                    out4[0:RB, :, :, c : c + 1].rearrange(
                                        "p g w k -> p g (w k)"
                                    ),
                                    mview,
                                    spv[c][0:RB, :, HALO + ox : HALO + ox + W],
                                )

                    for gi in range(G):
                        b = g0 + gi
                        nc.sync.dma_start(
                            out=y[b * H + r0 : b * H + r0 + RB, :],
                            in_=out_t[0:RB, gi * OSTR : gi * OSTR + R3],
                        )
    return nc


def _split_multiwait_drains(nc):
    """This walrus build accepts one sync wait per Drain (TPB_CTRL); split
    the Tile epilogue's multi-wait drains into single-wait chains."""
    import copy
    import bass_rust
    from concourse import mybir

    changed = False
    new_functions = []
    for function in nc.m.functions:
        new_function = copy.replace(function, blocks=[])
        new_function.set_allocations_from_list(function.allocations)
        for block in function.blocks:
            new_insts = []
            for ins in block.instructions:
                si = ins.sync_info
                if (
                    isinstance(ins, (mybir.InstDrain, mybir.InstNoOp))
                    and si is not None
                    and len(si.on_wait) > 1
                ):
                    changed = True
                    waits = list(si.on_wait)
                    for i, w in enumerate(waits[:-1]):
                        d = mybir.InstDrain(
                            name=f"{ins.name}_sw{i}", ins=[], outs=[],
                            bass_is_fusable=False,
                        )
                        d.engine = ins.engine
                        d.sync_info = bass_rust.SyncInfo(on_wait=[w], on_update=[])
                        new_insts.append(d)
                    ins.sync_info = bass_rust.SyncInfo(
                        on_wait=[waits[-1]], on_update=list(si.on_update)
                    )
                new_insts.append(ins)
            new_function.blocks.append(copy.replace(block, instructions=new_insts))
        new_functions.append(new_function)
    if changed:
        nc.m = copy.replace(nc.m, functions=new_functions)
    return nc


class _Runner:
    def __init__(self, nc, n_cores=8):
        import jax
        from jax.sharding import Mesh, PartitionSpec, NamedSharding
        from jax.experimental.shard_map import shard_map
        from concourse import mybir
        from concourse.bass2jax import (
            _bass_exec_p,
            install_neuronx_cc_hook,
            partition_id_tensor,
        )

        install_neuronx_cc_hook()
        if not nc.is_finalized():
            nc.finalize()
        _split_multiwait_drains(nc)

        self.jax = jax
        partition_name = (
            nc.partition_id_tensor.name if nc.partition_id_tensor else None
        )
        in_names, out_names, out_avals, zero_shapes = [], [], [], []
        for alloc in nc.m.functions[0].allocations:
            if not isinstance(alloc, mybir.MemoryLocationSet):
                continue
            name = alloc.memorylocations[0].name
            if alloc.kind == "ExternalInput":
                if name != partition_name:
                    in_names.append(name)
            elif alloc.kind == "ExternalOutput":
                out_names.append(name)
                shape = tuple(alloc.tensor_shape)
                dtype = mybir.dt.np(alloc.dtype)
                out_avals.append(jax.core.ShapedArray(shape, dtype))
                zero_shapes.append((shape, dtype))
        n_params = len(in_names)
        n_outs = len(out_avals)
        all_in_names = list(in_names) + list(out_names)
        if partition_name is not None:
            all_in_names.append(partition_name)
        donate = tuple(range(n_params, n_params + n_outs))

        def _body(*args):
            operands = list(args)
            if partition_name is not None:
                operands.append(partition_id_tensor())
            outs = _bass_exec_p.bind(
                *operands,
                out_avals=tuple(out_avals),
                in_names=tuple(all_in_names),
                out_names=tuple(out_names),
                lowering_input_output_aliases=(),
                sim_require_finite=True,
                sim_require_nnan=True,
                nc=nc,
            )
            return tuple(outs)

        devices = jax.devices()[:n_cores]
        mesh = Mesh(np.asarray(devices), ("core",))
        in_specs = (PartitionSpec("core"),) * (n_params + n_outs)
        out_specs = (PartitionSpec("core"),) * n_outs
        self.sharded = jax.jit(
            shard_map(
                _body, mesh=mesh, in_specs=in_specs, out_specs=out_specs,
                check_rep=False,
            ),
            donate_argnums=donate,
            keep_unused=True,
        )
        self.devices = devices
        self.shard = NamedSharding(mesh, PartitionSpec("core"))
        self.in_names, self.out_names = in_names, out_names
        self.out_avals, self.zero_shapes = out_avals, zero_shapes
        self.n_cores = n_cores
        # y-init buffers: uploaded once, then recycled from the previous
        # call's outputs (the kernel fully overwrites y; donation consumes
        # the buffers each call)
        self._ybufs = None

    def prep_inputs(self, in_maps):
        """Upload per-core shards in parallel (one device_put per device)."""
        from concurrent.futures import ThreadPoolExecutor

        jax = self.jax
        arrays = []
        for name in self.in_names:
            shards = [np.asarray(m[name]) for m in in_maps]
            full_shape = (
                self.n_cores * shards[0].shape[0],
                *shards[0].shape[1:],
            )
            with ThreadPoolExecutor(self.n_cores) as ex:
                parts = list(
                    ex.map(
                        lambda t: jax.device_put(t[0], t[1]),
                        zip(shards, self.devices),
                    )
                )
            arrays.append(
                jax.make_array_from_single_device_arrays(
                    full_shape, self.shard, parts
                )
            )
        jax.block_until_ready(arrays)
        return arrays

    def _get_ybufs(self):
        if self._ybufs is None:
            jax = self.jax
            zs = [
                jax.device_put(
                    np.zeros((self.n_cores * s[0], *s[1:]), d), self.shard
                )
                for (s, d) in self.zero_shapes
            ]
            jax.block_until_ready(zs)
            self._ybufs = zs
        return self._ybufs

    def run(self, dev_in):
        ybufs = self._get_ybufs()
        # the y-init buffers are donated (consumed) by this call; drop them
        # first so a failure cannot leave consumed arrays cached
        self._ybufs = None
        out = self.sharded(*dev_in, *ybufs)
        self.jax.block_until_ready(out)
        self._ybufs = list(out)
        return out

    def fetch(self, out):
        """Download the output shards in parallel."""
        from concurrent.futures import ThreadPoolExecutor

        res = []
        for arr in out:
            shards = sorted(
                arr.addressable_shards, key=lambda s: s.index[0].start
            )
            with ThreadPoolExecutor(self.n_cores) as ex:
                parts = list(ex.map(lambda s: np.asarray(s.data), shards))
            res.append(parts)
        return res

    def run_maps(self, in_maps):
        parts = self.fetch(self.run(self.prep_inputs(in_maps)))
        return [
            {name: parts[i][c] for i, name in enumerate(self.out_names)}
            for c in range(self.n_cores)
        ]


def _get_runner():
    if "r" not in _CACHE:
        _CACHE["r"] = _Runner(_build_module())
    return _CACHE["r"]


def _host_code(dx, dy):
    """Window-offset code per pixel, bit-exact vs the CPU jax reference.
    Returns (code uint8, ok bool)."""
    H, W = _H, _W
    cols = np.arange(W, dtype=np.float32)[None, None, :]
    rows = np.arange(H, dtype=np.float32)[None, :, None]

    def off(t, lim, base_i):
        # wrap into [0, lim] exactly as f32 mod does, floor via int cast,
        # clamp the t==lim boundary, then window-normalize around base
        tw = np.where(
            t < 0, t + np.float32(lim),
            np.where(t >= lim, t - np.float32(lim), t),
        ).astype(np.int32)
        o = np.minimum(tw, lim - 1) - base_i
        o = np.where(o > 112, o - lim, o)
        return np.where(o < -112, o + lim, o)

    offx = off(cols + dx, W, np.arange(W, dtype=np.int32)[None, None, :])
    offy = off(rows + dy, H, np.arange(H, dtype=np.int32)[None, :, None])
    ok = bool(
        offx.min() >= -_HALO and offx.max() <= _HALO
        and offy.min() >= -_HALO and offy.max() <= _HALO
    )
    if not ok:
        return None, False
    codes = ((offy + _HALO) * _WIN + (offx + _HALO)).astype(np.uint8)
    return codes, True


def _kernel_np(x):
    """Exact reference semantics (including jax's clamp of the f32 mod
    boundary case) — robustness fallback."""
    H, W = _H, _W
    img = x[..., 0:3]
    dx = x[..., 3]
    dy = x[..., 4]
    cols = np.arange(W, dtype=np.float32)
    rows = np.arange(H, dtype=np.float32)[:, None]
    Xi = np.minimum(
        np.mod(cols[None, None, :] + dx, np.float32(W)).astype(np.int32), W - 1
    )
    Yi = np.minimum(
        np.mod(rows[None, :, :] + dy, np.float32(H)).astype(np.int32), H - 1
    )
    b = np.arange(x.shape[0])[:, None, None]
    return img[b, Yi, Xi]


def _kernel_jax_device(x):
    """Tier-2 fallback: run the warp gather on the 8 NeuronCores via
    XLA-Neuron's native gather path."""
    import jax
    import jax.numpy as jnp

    H, W = _H, _W

    def body(xs):  # [B, H, W, 5] per device
        img = xs[..., 0:3]
        dx = xs[..., 3]
        dy = xs[..., 4]
        cols = jnp.arange(W, dtype=jnp.float32)
        rows = jnp.arange(H, dtype=jnp.float32)[:, None]
        Xi = jnp.mod(cols[None, None, :] + dx, float(W)).astype(jnp.int32)
        Yi = jnp.mod(rows[None, :, :] + dy, float(H)).astype(jnp.int32)
        b = jnp.arange(xs.shape[0])[:, None, None]
        return img[b, Yi, Xi]

    if "jdk" not in _CACHE:
        _CACHE["jdk"] = jax.jit(body)
    f = _CACHE["jdk"]
    devices = jax.devices()[:8]
    shards = x.reshape(8, _B, H, W, 5)
    dev_in = [jax.device_put(shards[i], devices[i]) for i in range(8)]
    outs = [f(s) for s in dev_in]
    host = jax.device_get(outs)
    return np.concatenate(host, axis=0)


_USE_BASS = True


def _kernel_bass(x):
    """Optimized flow: start the (async) img upload first, compute the
    code on the host while it streams, then upload the code, run, and
    upcast during the parallel fetch."""
    from concurrent.futures import ThreadPoolExecutor
    import ml_dtypes
    import jax

    from concurrent.futures import ThreadPoolExecutor as _TPE

    n_cores = 8
    r = _get_runner()

    # pipeline: each shard is converted to bf16 in a worker thread and its
    # (async) upload issued immediately; the window code is computed on the
    # main thread while the uploads stream
    x8 = x.reshape(n_cores, _B, _H, _W, 5)

    def conv_put(c):
        a = np.ascontiguousarray(x8[c, ..., 0:3]).astype(ml_dtypes.bfloat16)
        return jax.device_put(a.reshape(_B * _H, _W * 3), r.devices[c])

    pool = _TPE(n_cores)
    img_futs = [pool.submit(conv_put, c) for c in range(n_cores)]

    codes, ok = _host_code(x[..., 3], x[..., 4])
    if not ok:
        pool.shutdown(wait=True)
        raise ValueError("displacement exceeds the 13x13 window")
    code_sh = codes.reshape(n_cores, _B * _H, _W)
    code_parts = [
        jax.device_put(code_sh[c], r.devices[c]) for c in range(n_cores)
    ]
    img_parts = [f.result() for f in img_futs]
    pool.shutdown(wait=False)

    by_name = {
        "img": (img_parts, (n_cores * _B * _H, _W * 3)),
        "code": (code_parts, (n_cores * _B * _H, _W)),
    }
    arrays = [
        jax.make_array_from_single_device_arrays(
            by_name[name][1], r.shard, by_name[name][0]
        )
        for name in r.in_names
    ]
    jax.block_until_ready(arrays)

    out = r.run(arrays)[0]
    shards = sorted(out.addressable_shards, key=lambda s: s.index[0].start)
    y = np.empty((128, _H, _W, 3), np.float32)
    yv = y.reshape(n_cores, _B * _H, _W * 3)

    def fetch_one(ci):
        c, s = ci
        yv[c] = np.asarray(s.data).astype(np.float32)

    with ThreadPoolExecutor(n_cores) as ex:
        list(ex.map(fetch_one, enumerate(shards)))
    return y


def kernel(x):
    x = np.ascontiguousarray(np.asarray(x, dtype=np.float32))
    assert x.shape == (128, _H, _W, 5), x.shape
    if _USE_BASS:
        try:
            return _kernel_bass(x)
        except Exception as e:
            sys.stderr.write(
                f"kernel: bass path failed ({e!r}); jax-device fallback\n"
            )
    try:
        return _kernel_jax_device(x)
    except Exception as e:
        sys.stderr.write(f"kernel: jax-device failed ({e!r}); numpy fallback\n")
        return _kernel_np(x)


# revision 25
# speedup vs baseline: 1.5578x; 1.0259x over previous
"""Trainium2 Bass kernel for nn_Bilinear_70222715290053.

Problem: x [128, 224, 224, 5] f32 where channels 0:3 are an image and
channels 3,4 are per-pixel displacements (dx, dy). Output [128,224,224,3]:
  out[b,i,j,:] = img[b, int(mod(i+dy, 224)), int(mod(j+dx, 224)), :]

Key property: dx, dy ~ N(0,1), so |displacement| <= ~5.5 — the gather is a
LOCAL warp within a 13x13 window (kernel() verifies the bound at runtime
and falls back to an exact path if violated).

System design (the host<->device axon tunnel runs at ~40MB/s and is the
end-to-end bottleneck, so minimize bytes moved):
  - Host side (cheap, vectorized, bit-exact vs the CPU jax reference —
    verified): compute the source-pixel window offsets and fold them into
    one uint8 code = (offy+6)*13 + (offx+6) per pixel [6.4MB], and convert
    the 3 image channels to bf16 [38.6MB]. Upload 45MB instead of 128MB.
  - Device side (Bass, SPMD on 8 cores, batch-sharded 16 images/core):
    rows live in the partition dim; a round processes G=4 images x 112
    output rows (+6 halo rows each side via extra partitions, mod-224
    wrapped). 169 select terms: mask = is_equal(code, t) on the DVE, then
    3 copy_predicated moves from the (oy, ox)-shifted x-padded channel
    planes into the output tile. Partition shifts are materialized by
    cheap SBUF->SBUF DMAs (compute engines can only address partitions
    0/32/64/96; DMA has no such limit). Every pixel matches exactly one
    term.
  - Output returns as bf16 [38.6MB instead of 77MB]; host upcasts to f32.
    Total quantization error is one bf16 rounding: rel err ~4e-3 (< 2e-2).

Self-contained: builds the Bass module, compiles through neuronx_cc via the
bass2jax custom call, and runs SPMD on 8 NeuronCores via shard_map.
"""

import sys

sys.path.insert(0, "/opt/trn_rl_repo")

import numpy as np

_CACHE = {}

_B, _H, _W = 16, 224, 224  # per-core shard
_HALO_N = 6   # max negative offset (toward smaller index)
_HALO_P = 5   # max positive offset (floor(j+d)-j <= 5 for |d| < 6)
_WIN = _HALO_N + _HALO_P + 1  # 12
_G = 4  # images per round


def _build_module(B=_B, H=_H, W=_W, G=_G, HN=_HALO_N, HP=_HALO_P):
    from concourse import mybir, bacc
    import concourse.tile as tile

    BF16 = mybir.dt.bfloat16
    F16 = mybir.dt.float16
    I16 = mybir.dt.int16
    U8 = mybir.dt.uint8
    Alu = mybir.AluOpType

    NB = 2               # row blocks per image
    RB = H // NB         # 112 output rows per block
    WPAD = W + HN + HP   # 235 x-padded plane width
    WIN = HN + HP + 1    # 12
    R3 = W * 3
    # Band partition layout: 0..RB-1 = central rows r0..r0+RB-1,
    # RB..RB+HN-1 = top halo rows r0-HN..r0-1 (mod H),
    # RB+HN..RB+HN+HP-1 = bottom halo rows r0+RB..+RB+HP-1 (mod H).
    PT = RB
    PB = RB + HN
    BAND = RB + HN + HP

    nc = bacc.Bacc(None, target_bir_lowering=False)
    img = nc.declare_dram_parameter("img", [B * H, R3], BF16, isOutput=False)
    code = nc.declare_dram_parameter("code", [B * H, W], U8, isOutput=False)
    y = nc.declare_dram_parameter("y", [B * H, R3], BF16, isOutput=True)

    with tile.TileContext(nc) as tc:
        with (
            tc.tile_pool(name="rec", bufs=2) as rpool,
            tc.tile_pool(name="planes", bufs=2) as ppool,
            tc.tile_pool(name="shift", bufs=2) as spool,
            tc.tile_pool(name="tmp", bufs=2) as tpool,
            tc.tile_pool(name="outp", bufs=2) as opool,
        ):
            for g0 in range(0, B, G):
                for blk in range(NB):
                    r0 = blk * RB
                    rtop = (r0 - HN) % H
                    rbot = (r0 + RB) % H
                    band = rpool.tile([128, G * R3], BF16, tag="band")
                    codet = tpool.tile([128, G * W], U8, tag="codet")
                    for gi in range(G):
                        b = g0 + gi
                        fs = slice(gi * R3, (gi + 1) * R3)
                        nc.sync.dma_start(
                            out=band[0:RB, fs],
                            in_=img[b * H + r0 : b * H + r0 + RB, :],
                        )
                        nc.sync.dma_start(
                            out=band[PT : PT + HN, fs],
                            in_=img[b * H + rtop : b * H + rtop + HN, :],
                        )
                        nc.sync.dma_start(
                            out=band[PB : PB + HP, fs],
                            in_=img[b * H + rbot : b * H + rbot + HP, :],
                        )
                        nc.sync.dma_start(
                            out=codet[0:RB, gi * W : (gi + 1) * W],
                            in_=code[b * H + r0 : b * H + r0 + RB, :],
                        )

                    band4 = band[:].rearrange("p (g w c) -> p g w c", g=G, c=3)

                    # x-wrap-padded bf16 channel planes [BAND, G, WPAD]
                    planes = []
                    for c in range(3):
                        pc = ppool.tile([128, G * WPAD], BF16, tag=f"plane{c}")
                        pc3 = pc[:].rearrange("p (g w) -> p g w", g=G)
                        nc.vector.tensor_copy(
                            out=pc3[0:BAND, :, HN : HN + W],
                            in_=band4[0:BAND, :, :, c : c + 1].rearrange(
                                "p g w k -> p g (w k)"
                            ),
                        )
                        nc.vector.tensor_copy(
                            out=pc3[0:BAND, :, 0:HN],
                            in_=band4[
                                0:BAND, :, W - HN : W, c : c + 1
                            ].rearrange("p g w k -> p g (w k)"),
                        )
                        nc.vector.tensor_copy(
                            out=pc3[0:BAND, :, HN + W : WPAD],
                            in_=band4[0:BAND, :, 0:HP, c : c + 1].rearrange(
                                "p g w k -> p g (w k)"
                            ),
                        )
                        planes.append(pc)

                    def s(t):
                        return t[0:RB, :]

                    # code as fp16 so the 169 is_equal passes run in the
                    # DVE 16-bit fast mode
                    codeh = tpool.tile([128, G * W], F16, tag="codeh")
                    nc.vector.tensor_copy(out=s(codeh), in_=s(codet))

                    # Pad per-image strides of mask/out so their interp views
                    # cannot dim-merge (copy_predicated needs all three
                    # operand views shaped identically (RB, G, W)).
                    OSTR = R3 + 16
                    MSTR = W + 16
                    out_t = opool.tile([128, G * OSTR], BF16, tag="out")
                    out4 = (
                        out_t[:]
                        .rearrange("p (g q) -> p g q", g=G)[:, :, 0:R3]
                        .rearrange("p g (w c) -> p g w c", c=3)
                    )
                    maskh = tpool.tile([128, G * MSTR], I16, tag="maskh")
                    mview = maskh[:].rearrange("p (g q) -> p g q", g=G)[
                        0:RB, :, 0:W
                    ]
                    ch3 = codeh[:].rearrange("p (g w) -> p g w", g=G)

                    for oy in range(-HN, HP + 1):
                        # partition-shifted plane copies (DMA may start at
                        # any partition; compute engines may not). Band
                        # layout makes each shift at most 2 contiguous
                        # pieces.
                        if oy == 0:
                            sps = planes
                        else:
                            sps = []
                            for c in range(3):
                                sp = spool.tile(
                                    [128, G * WPAD], BF16, tag=f"sp{c}"
                                )
                                if oy < 0:
                                    k = -oy
                                    nc.sync.dma_start(
                                        out=sp[0:k, :],
                                        in_=planes[c][PB - k : PB, :],
                                    )
                                    nc.sync.dma_start(
                                        out=sp[k:RB, :],
                                        in_=planes[c][0 : RB - k, :],
                                    )
                                else:
                                    nc.sync.dma_start(
                                        out=sp[0 : RB - oy, :],
                                        in_=planes[c][oy:RB, :],
                                    )
                                    nc.sync.dma_start(
                                        out=sp[RB - oy : RB, :],
                                        in_=planes[c][PB : PB + oy, :],
                                    )
                                sps.append(sp)
                        spv = [
                            t[:].rearrange("p (g w) -> p g w", g=G) for t in sps
                        ]
                        for ox in range(-HN, HP + 1):
                            t_code = float((oy + HN) * WIN + (ox + HN))
                            nc.vector.tensor_scalar(
                                out=mview, in0=ch3[0:RB],
                                scalar1=t_code, scalar2=None,
                                op0=Alu.is_equal,
                            )
                            for c in range(3):
                                nc.vector.copy_predicated(
                                    out4[0:RB, :, :, c : c + 1].rearrange(
                                        "p g w k -> p g (w k)"
                                    ),
                                    mview,
                                    spv[c][0:RB, :, HN + ox : HN + ox + W],
                                )

                    for gi in range(G):
                        b = g0 + gi
                        nc.sync.dma_start(
                            out=y[b * H + r0 : b * H + r0 + RB, :],
                            in_=out_t[0:RB, gi * OSTR : gi * OSTR + R3],
                        )
    return nc


def _split_multiwait_drains(nc):
    """This walrus build accepts one sync wait per Drain (TPB_CTRL); split
    the Tile epilogue's multi-wait drains into single-wait chains."""
    import copy
    import bass_rust
    from concourse import mybir

    changed = False
    new_functions = []
    for function in nc.m.functions:
        new_function = copy.replace(function, blocks=[])
        new_function.set_allocations_from_list(function.allocations)
        for block in function.blocks:
            new_insts = []
            for ins in block.instructions:
                si = ins.sync_info
                if (
                    isinstance(ins, (mybir.InstDrain, mybir.InstNoOp))
                    and si is not None
                    and len(si.on_wait) > 1
                ):
                    changed = True
                    waits = list(si.on_wait)
                    for i, w in enumerate(waits[:-1]):
                        d = mybir.InstDrain(
                            name=f"{ins.name}_sw{i}", ins=[], outs=[],
                            bass_is_fusable=False,
                        )
                        d.engine = ins.engine
                        d.sync_info = bass_rust.SyncInfo(on_wait=[w], on_update=[])
                        new_insts.append(d)
                    ins.sync_info = bass_rust.SyncInfo(
                        on_wait=[waits[-1]], on_update=list(si.on_update)
                    )
                new_insts.append(ins)
            new_function.blocks.append(copy.replace(block, instructions=new_insts))
        new_functions.append(new_function)
    if changed:
        nc.m = copy.replace(nc.m, functions=new_functions)
    return nc


class _Runner:
    def __init__(self, nc, n_cores=8):
        import jax
        from jax.sharding import Mesh, PartitionSpec, NamedSharding
        from jax.experimental.shard_map import shard_map
        from concourse import mybir
        from concourse.bass2jax import (
            _bass_exec_p,
            install_neuronx_cc_hook,
            partition_id_tensor,
        )

        install_neuronx_cc_hook()
        if not nc.is_finalized():
            nc.finalize()
        _split_multiwait_drains(nc)

        self.jax = jax
        partition_name = (
            nc.partition_id_tensor.name if nc.partition_id_tensor else None
        )
        in_names, out_names, out_avals, zero_shapes = [], [], [], []
        for alloc in nc.m.functions[0].allocations:
            if not isinstance(alloc, mybir.MemoryLocationSet):
                continue
            name = alloc.memorylocations[0].name
            if alloc.kind == "ExternalInput":
                if name != partition_name:
                    in_names.append(name)
            elif alloc.kind == "ExternalOutput":
                out_names.append(name)
                shape = tuple(alloc.tensor_shape)
                dtype = mybir.dt.np(alloc.dtype)
                out_avals.append(jax.core.ShapedArray(shape, dtype))
                zero_shapes.append((shape, dtype))
        n_params = len(in_names)
        n_outs = len(out_avals)
        all_in_names = list(in_names) + list(out_names)
        if partition_name is not None:
            all_in_names.append(partition_name)
        donate = tuple(range(n_params, n_params + n_outs))

        def _body(*args):
            operands = list(args)
            if partition_name is not None:
                operands.append(partition_id_tensor())
            outs = _bass_exec_p.bind(
                *operands,
                out_avals=tuple(out_avals),
                in_names=tuple(all_in_names),
                out_names=tuple(out_names),
                lowering_input_output_aliases=(),
                sim_require_finite=True,
                sim_require_nnan=True,
                nc=nc,
            )
            return tuple(outs)

        devices = jax.devices()[:n_cores]
        mesh = Mesh(np.asarray(devices), ("core",))
        in_specs = (PartitionSpec("core"),) * (n_params + n_outs)
        out_specs = (PartitionSpec("core"),) * n_outs
        self.sharded = jax.jit(
            shard_map(
                _body, mesh=mesh, in_specs=in_specs, out_specs=out_specs,
                check_rep=False,
            ),
            donate_argnums=donate,
            keep_unused=True,
        )
        self.devices = devices
        self.shard = NamedSharding(mesh, PartitionSpec("core"))
        self.in_names, self.out_names = in_names, out_names
        self.out_avals, self.zero_shapes = out_avals, zero_shapes
        self.n_cores = n_cores
        # y-init buffers: uploaded once, then recycled from the previous
        # call's outputs (the kernel fully overwrites y; donation consumes
        # the buffers each call)
        self._ybufs = None

    def prep_inputs(self, in_maps):
        """Upload per-core shards in parallel (one device_put per device)."""
        from concurrent.futures import ThreadPoolExecutor

        jax = self.jax
        arrays = []
        for name in self.in_names:
            shards = [np.asarray(m[name]) for m in in_maps]
            full_shape = (
                self.n_cores * shards[0].shape[0],
                *shards[0].shape[1:],
            )
            with ThreadPoolExecutor(self.n_cores) as ex:
                parts = list(
                    ex.map(
                        lambda t: jax.device_put(t[0], t[1]),
                        zip(shards, self.devices),
                    )
                )
            arrays.append(
                jax.make_array_from_single_device_arrays(
                    full_shape, self.shard, parts
                )
            )
        jax.block_until_ready(arrays)
        return arrays

    def _get_ybufs(self):
        if self._ybufs is None:
            jax = self.jax
            zs = [
                jax.device_put(
                    np.zeros((self.n_cores * s[0], *s[1:]), d), self.shard
                )
                for (s, d) in self.zero_shapes
            ]
            jax.block_until_ready(zs)
            self._ybufs = zs
        return self._ybufs

    def run(self, dev_in):
        ybufs = self._get_ybufs()
        # the y-init buffers are donated (consumed) by this call; drop them
        # first so a failure cannot leave consumed arrays cached
        self._ybufs = None
        out = self.sharded(*dev_in, *ybufs)
        self.jax.block_until_ready(out)
        self._ybufs = list(out)
        return out

    def fetch(self, out):
        """Download the output shards in parallel."""
        from concurrent.futures import ThreadPoolExecutor

        res = []
        for arr in out:
            shards = sorted(
                arr.addressable_shards, key=lambda s: s.index[0].start
            )
            with ThreadPoolExecutor(self.n_cores) as ex:
                parts = list(ex.map(lambda s: np.asarray(s.data), shards))
            res.append(parts)
        return res

    def run_maps(self, in_maps):
        parts = self.fetch(self.run(self.prep_inputs(in_maps)))
        return [
            {name: parts[i][c] for i, name in enumerate(self.out_names)}
            for c in range(self.n_cores)
        ]


def _get_runner():
    if "r" not in _CACHE:
        _CACHE["r"] = _Runner(_build_module())
    return _CACHE["r"]


def _host_code(dx, dy):
    """Window-offset code per pixel, bit-exact vs the CPU jax reference.
    Returns (code uint8, ok bool)."""
    H, W = _H, _W
    cols = np.arange(W, dtype=np.float32)[None, None, :]
    rows = np.arange(H, dtype=np.float32)[None, :, None]

    def off(t, lim, base_i):
        # wrap into [0, lim] exactly as f32 mod does, floor via int cast,
        # clamp the t==lim boundary, then window-normalize around base
        tw = np.where(
            t < 0, t + np.float32(lim),
            np.where(t >= lim, t - np.float32(lim), t),
        ).astype(np.int32)
        o = np.minimum(tw, lim - 1) - base_i
        o = np.where(o > 112, o - lim, o)
        return np.where(o < -112, o + lim, o)

    offx = off(cols + dx, W, np.arange(W, dtype=np.int32)[None, None, :])
    offy = off(rows + dy, H, np.arange(H, dtype=np.int32)[None, :, None])
    ok = bool(
        offx.min() >= -_HALO_N and offx.max() <= _HALO_P
        and offy.min() >= -_HALO_N and offy.max() <= _HALO_P
    )
    if not ok:
        return None, False
    codes = ((offy + _HALO_N) * _WIN + (offx + _HALO_N)).astype(np.uint8)
    return codes, True


def _kernel_np(x):
    """Exact reference semantics (including jax's clamp of the f32 mod
    boundary case) — robustness fallback."""
    H, W = _H, _W
    img = x[..., 0:3]
    dx = x[..., 3]
    dy = x[..., 4]
    cols = np.arange(W, dtype=np.float32)
    rows = np.arange(H, dtype=np.float32)[:, None]
    Xi = np.minimum(
        np.mod(cols[None, None, :] + dx, np.float32(W)).astype(np.int32), W - 1
    )
    Yi = np.minimum(
        np.mod(rows[None, :, :] + dy, np.float32(H)).astype(np.int32), H - 1
    )
    b = np.arange(x.shape[0])[:, None, None]
    return img[b, Yi, Xi]


def _kernel_jax_device(x):
    """Tier-2 fallback: run the warp gather on the 8 NeuronCores via
    XLA-Neuron's native gather path."""
    import jax
    import jax.numpy as jnp

    H, W = _H, _W

    def body(xs):  # [B, H, W, 5] per device
        img = xs[..., 0:3]
        dx = xs[..., 3]
        dy = xs[..., 4]
        cols = jnp.arange(W, dtype=jnp.float32)
        rows = jnp.arange(H, dtype=jnp.float32)[:, None]
        Xi = jnp.mod(cols[None, None, :] + dx, float(W)).astype(jnp.int32)
        Yi = jnp.mod(rows[None, :, :] + dy, float(H)).astype(jnp.int32)
        b = jnp.arange(xs.shape[0])[:, None, None]
        return img[b, Yi, Xi]

    if "jdk" not in _CACHE:
        _CACHE["jdk"] = jax.jit(body)
    f = _CACHE["jdk"]
    devices = jax.devices()[:8]
    shards = x.reshape(8, _B, H, W, 5)
    dev_in = [jax.device_put(shards[i], devices[i]) for i in range(8)]
    outs = [f(s) for s in dev_in]
    host = jax.device_get(outs)
    return np.concatenate(host, axis=0)


_USE_BASS = True


def _kernel_bass(x):
    """Optimized flow: start the (async) img upload first, compute the
    code on the host while it streams, then upload the code, run, and
    upcast during the parallel fetch."""
    from concurrent.futures import ThreadPoolExecutor
    import ml_dtypes
    import jax

    from concurrent.futures import ThreadPoolExecutor as _TPE

    n_cores = 8
    r = _get_runner()

    # pipeline: each shard is converted to bf16 in a worker thread and its
    # (async) upload issued immediately; the window code is computed on the
    # main thread while the uploads stream
    x8 = x.reshape(n_cores, _B, _H, _W, 5)

    def conv_put(c):
        a = np.ascontiguousarray(x8[c, ..., 0:3]).astype(ml_dtypes.bfloat16)
        return jax.device_put(a.reshape(_B * _H, _W * 3), r.devices[c])

    pool = _TPE(n_cores)
    img_futs = [pool.submit(conv_put, c) for c in range(n_cores)]

    codes, ok = _host_code(x[..., 3], x[..., 4])
    if not ok:
        pool.shutdown(wait=True)
        raise ValueError("displacement exceeds the select window")
    code_sh = codes.reshape(n_cores, _B * _H, _W)
    code_parts = [
        jax.device_put(code_sh[c], r.devices[c]) for c in range(n_cores)
    ]
    img_parts = [f.result() for f in img_futs]
    pool.shutdown(wait=False)

    by_name = {
        "img": (img_parts, (n_cores * _B * _H, _W * 3)),
        "code": (code_parts, (n_cores * _B * _H, _W)),
    }
    arrays = [
        jax.make_array_from_single_device_arrays(
            by_name[name][1], r.shard, by_name[name][0]
        )
        for name in r.in_names
    ]
    jax.block_until_ready(arrays)

    # dispatch without an explicit completion round-trip; the per-shard
    # fetches queue behind execution client-side, saving ~1 tunnel RTT
    ybufs = r._get_ybufs()
    r._ybufs = None
    out_all = r.sharded(*arrays, *ybufs)
    r._ybufs = list(out_all)
    out = out_all[0]
    shards = sorted(out.addressable_shards, key=lambda s: s.index[0].start)
    for s in shards:
        try:
            s.data.copy_to_host_async()
        except Exception:
            pass
    y = np.empty((128, _H, _W, 3), np.float32)
    yv = y.reshape(n_cores, _B * _H, _W * 3)

    def fetch_one(ci):
        c, s = ci
        yv[c] = np.asarray(s.data).astype(np.float32)

    with ThreadPoolExecutor(n_cores) as ex:
        list(ex.map(fetch_one, enumerate(shards)))
    return y


def kernel(x):
    x = np.ascontiguousarray(np.asarray(x, dtype=np.float32))
    assert x.shape == (128, _H, _W, 5), x.shape
    if _USE_BASS:
        try:
            return _kernel_bass(x)
        except Exception as e:
            sys.stderr.write(
                f"kernel: bass path failed ({e!r}); jax-device fallback\n"
            )
    try:
        return _kernel_jax_device(x)
    except Exception as e:
        sys.stderr.write(f"kernel: jax-device failed ({e!r}); numpy fallback\n")
        return _kernel_np(x)
